# revision 1
# baseline (speedup 1.0000x reference)
"""Trainium2 Bass kernel for nn_MultiHeadDistanceLayer (sparse_attention).

Math: for each (head h, batch b) the reference collapses to
    S[m] = sum_k attn[k-m, k] * w[k],   w[k] = sigmoid(x @ Wv)[L-1-k, h]
(weighted superdiagonal sums of the attention matrix), followed by a
window-3 same-padded average pool over m (padding excluded from the
divisor):  out[b, m, h] = (S[m-1] + S[m] + S[m+1]) / cnt[m].

Sharding: 8 heads -> 8 NeuronCores; each core computes its head for both
batches.  Per (h, b) the kernel runs flash-style over 16 row blocks of 128
queries: scores via PE (K=HD=32), exp on ACT (with free row-sum accum ->
softmax denominators d), gating by w on DVE, then a *skewed* DMA write of
the probability block to a DRAM scratch so that superdiagonal m lands at
column m of every row.  A plain strided read back + a [1/d]-weighted
ones-matmul on PE reduces partitions, PSUM-accumulating S[m] across
blocks.  The window-3 pool runs on-device in 512-chunks pipelined behind
the S accumulation; host only slices inputs / stacks the per-core (B, L)
outputs into (B, L, H).
"""

import contextlib

import numpy as np

import concourse.bacc as bacc
import concourse.bass as bass
import concourse.tile as tile
from concourse import mybir
from concourse.tile import add_dep_helper

B, L, D, H, HD, WIN = 2, 2048, 128, 8, 32, 3
NBLK = L // 128           # 16 row blocks per (h, b)
W = L + 128               # scratch row width (elements)
SCALE = float(HD) ** -0.5

FP16 = mybir.dt.float16
FP32 = mybir.dt.float32

DEFAULT_OPTS = dict(
    ew_bufs=10,
    rt_bufs=8,
    write_gpsimd=False,  # scratch writes via SWDGE (Pool) instead of HWDGE
    zero_bias=False,     # biases known to be zero -> plain copy instead of add
    skip_wmul=False,
    skip_write=False,
    skip_read=False,
    skip_reduce=False,
    skip_post=False,
    kt_act=True,         # prep K-copies on ACT (idle during prep)
    prep1_late=None,     # emit pair-1 prep after this many pair-0 blocks (None=upfront)
    reduce_delay=4,
    memset_dve=True,     # ew tail memset on DVE instead of Pool
    small_bufs=4,
    post_bufs=4,
    score_n1024=False,   # single N=1024 scores matmul per psum tile (fp16 moving)
    wmul_split=False,    # gate each exp half separately (shorter chain to the write)
)


def build_nc(repeat=1, **opts_kw):
    opts = dict(DEFAULT_OPTS, **opts_kw)
    nc = bacc.Bacc("TRN2", target_bir_lowering=False, debug=False)

    xpT = nc.dram_tensor("xpT", [B, D, L], FP16, kind="ExternalInput")
    xrevT = nc.dram_tensor("xrevT", [B, D, L], FP16, kind="ExternalInput")
    wq = nc.dram_tensor("wq", [D, HD], FP16, kind="ExternalInput")
    wk = nc.dram_tensor("wk", [D, HD], FP16, kind="ExternalInput")
    wv = nc.dram_tensor("wv", [D, 1], FP16, kind="ExternalInput")
    bqk = nc.dram_tensor("bqk", [2 * HD, 1], FP32, kind="ExternalInput")
    out = nc.dram_tensor("out", [B, L], FP32, kind="ExternalOutput")
    # one flat scratch region per (h, b) pair; row i of the logical [L, W]
    # grid holds the skew-shifted probability row i
    scr = [
        nc.dram_tensor(f"scr{b}", [L * W], FP16, kind="Internal") for b in range(B)
    ]

    with tile.TileContext(nc) as tc:
        with contextlib.ExitStack() as ctx:
            singles = ctx.enter_context(tc.tile_pool(name="singles", bufs=1))
            small = ctx.enter_context(tc.tile_pool(name="small", bufs=opts["small_bufs"]))
            ew_pool = ctx.enter_context(tc.tile_pool(name="ew", bufs=opts["ew_bufs"]))
            rt_pool = ctx.enter_context(tc.tile_pool(name="rt", bufs=opts["rt_bufs"]))
            ps_pool = ctx.enter_context(tc.tile_pool(name="ps", bufs=2, space="PSUM"))
            s_pool = ctx.enter_context(tc.tile_pool(name="spsum", bufs=1, space="PSUM"))
            post_pool = ctx.enter_context(tc.tile_pool(name="post", bufs=opts["post_bufs"]))

            # ---- constants (one-time) ----
            # ones row lives at partition 64 to match pv's col-group placement
            ones_row = singles.tile([97, 128], FP16)
            nc.vector.memset(ones_row, 1.0)
            cntr = singles.tile([1, L], FP32)
            nc.vector.memset(cntr, 1.0 / 3.0)
            nc.vector.memset(cntr[0:1, 0:1], 0.5)
            nc.vector.memset(cntr[0:1, L - 1 : L], 0.5)
            # preload ACT table sets (sigmoid first, exp second) while DMAs run
            warm = singles.tile([1, 8], FP32)
            nc.vector.memset(warm, 0.0)
            warm2 = singles.tile([1, 8], FP32)
            nc.scalar.activation(out=warm2, in_=warm,
                                 func=mybir.ActivationFunctionType.Sigmoid)

            prev_rd = {}

            def emit_prep_v(weights):
                """vrev rows: partition 64 = pair 0, partition 96 = pair 1."""
                _, _, _, wv_sb = weights
                xr = {}
                for b in range(B):
                    xr[b] = singles.tile([D, L], FP16, tag=f"xrevT{b}", name=f"xr{b}")
                    nc.sync.dma_start(out=xr[b][:, 0:1024], in_=xrevT[b, :, 0:1024])
                    nc.sync.dma_start(out=xr[b][:, 1024:L], in_=xrevT[b, :, 1024:L])
                vrev = singles.tile([97, L], FP16, tag="vrev")
                for half in range(2):
                    c0 = half * 1024
                    pv = ps_pool.tile([128, 1024], FP32, tag="ps")
                    for j in range(2):
                        for b in range(B):
                            nc.tensor.matmul(
                                out=pv[64 + 32 * b : 65 + 32 * b,
                                       j * 512 : (j + 1) * 512],
                                lhsT=wv_sb[:, :],
                                rhs=xr[b][:, c0 + j * 512 : c0 + (j + 1) * 512],
                                start=True, stop=True,
                                tile_position=(0, 64 + 32 * b),
                            )
                    for b in range(B):
                        nc.scalar.activation(
                            out=vrev[64 + 32 * b : 65 + 32 * b, c0 : c0 + 1024],
                            in_=pv[64 + 32 * b : 65 + 32 * b, :],
                            func=mybir.ActivationFunctionType.Sigmoid,
                        )
                return vrev

            def emit_prep(b, weights, vrev):
                """QT/KT [32, L] fp16, w_bcast [128, L] fp16 for pair b."""
                bqk_sb, wq_sb, wk_sb, wv_sb = weights
                xp_t = singles.tile([D, L], FP16, tag=f"xpT{b}")
                nc.sync.dma_start(out=xp_t[:, 0:1024], in_=xpT[b, :, 0:1024])
                nc.sync.dma_start(out=xp_t[:, 1024:L], in_=xpT[b, :, 1024:L])

                qt = singles.tile([HD, L], FP16, tag=f"QT{b}")
                kt = singles.tile([HD, L], FP16, tag=f"KT{b}")
                # packed prep psum: Q at partitions [0:32), K at [32:64);
                # copies emitted per 512-chunk so the psum slot frees early
                for half in range(2):
                    c0 = half * 1024
                    pqkv = ps_pool.tile([128, 1024], FP32, tag="ps")
                    for j in range(2):
                        cs = slice(c0 + j * 512, c0 + (j + 1) * 512)
                        js = slice(j * 512, (j + 1) * 512)
                        nc.tensor.matmul(
                            out=pqkv[0:HD, js], lhsT=wq_sb[:, :],
                            rhs=xp_t[:, cs], start=True, stop=True,
                        )
                        nc.tensor.matmul(
                            out=pqkv[HD : 2 * HD, js], lhsT=wk_sb[:, :],
                            rhs=xp_t[:, cs], start=True, stop=True,
                        )
                        if opts["zero_bias"]:
                            nc.vector.tensor_copy(
                                out=qt[:, cs], in_=pqkv[0:HD, js]
                            )
                            if opts["kt_act"]:
                                nc.scalar.copy(
                                    out=kt[:, cs], in_=pqkv[HD : 2 * HD, js]
                                )
                            else:
                                nc.vector.tensor_copy(
                                    out=kt[:, cs], in_=pqkv[HD : 2 * HD, js]
                                )
                        else:
                            nc.vector.tensor_scalar_add(
                                out=qt[:, cs],
                                in0=pqkv[0:HD, js],
                                scalar1=bqk_sb[0:HD],
                            )
                            if opts["kt_act"]:
                                nc.scalar.add(
                                    out=kt[:, cs],
                                    in_=pqkv[HD : 2 * HD, js],
                                    add=bqk_sb[HD : 2 * HD],
                                )
                            else:
                                nc.vector.tensor_scalar_add(
                                    out=kt[:, cs],
                                    in0=pqkv[HD : 2 * HD, js],
                                    scalar1=bqk_sb[HD : 2 * HD],
                                )
                # broadcast vrev across 128 partitions via K=1 matmul
                wb = singles.tile([128, L], FP16, tag=f"WB{b}")
                for half in range(2):
                    c0 = half * 1024
                    pb = ps_pool.tile([128, 1024], FP32, tag="ps")
                    for j in range(2):
                        nc.tensor.matmul(
                            out=pb[:, j * 512 : (j + 1) * 512],
                            lhsT=ones_row[64 + 32 * b : 65 + 32 * b, :],
                            rhs=vrev[64 + 32 * b : 65 + 32 * b,
                                     c0 + j * 512 : c0 + (j + 1) * 512],
                            start=True,
                            stop=True,
                            tile_position=(64 + 32 * b, 0),
                        )
                    nc.vector.tensor_copy(out=wb[:, c0 : c0 + 1024], in_=pb[:, :])
                return qt, kt, wb

            def emit_instance():
                bqk_sb = singles.tile([2 * HD, 1], FP32, tag="bqk_sb")
                nc.sync.dma_start(out=bqk_sb, in_=bqk[:, :])
                wq_sb = singles.tile([D, HD], FP16, tag="wq_sb")
                nc.sync.dma_start(out=wq_sb, in_=wq[:, :])
                wk_sb = singles.tile([D, HD], FP16, tag="wk_sb")
                nc.sync.dma_start(out=wk_sb, in_=wk[:, :])
                wv_sb = singles.tile([D, 1], FP16, tag="wv_sb")
                nc.sync.dma_start(out=wv_sb, in_=wv[:, :])
                weights = (bqk_sb, wq_sb, wk_sb, wv_sb)

                QT, KT, WB = {}, {}, {}
                REDUCE_DELAY = opts["reduce_delay"]

                # all sigmoids first (single table-set load), then exp-table
                # warm-up, then the ACT-free Q/K preps
                vrev_all = emit_prep_v(weights)
                VR = {b: vrev_all for b in range(B)}
                warmx = singles.tile([1, 8], FP32, tag="warmx")
                nc.scalar.activation(out=warmx, in_=vrev_all[64:65, 0:8],
                                     func=mybir.ActivationFunctionType.Exp)

                prep1_late = opts["prep1_late"]
                QT[0], KT[0], WB[0] = emit_prep(0, weights, VR[0])
                if prep1_late is None:
                    QT[1], KT[1], WB[1] = emit_prep(1, weights, VR[1])

                for b in range(B):
                    s_acc = s_pool.tile([1, L], FP32, tag="S")
                    res = post_pool.tile([1, L], FP32, tag="res")
                    pending = []
                    done_chunks = set()

                    def emit_reduce(item, s_acc=s_acc):
                        rr, rt_t, dr16, rlen_r = item
                        for j in range(4):
                            m0 = j * 512
                            wj = min(512, rlen_r - m0)
                            if wj <= 0:
                                continue
                            nc.tensor.matmul(
                                out=s_acc[0:1, m0 : m0 + wj],
                                lhsT=dr16[:, 0:1],
                                rhs=rt_t[:, m0 : m0 + wj],
                                start=(rr == 0),
                                stop=(rr == 15 - 4 * j),
                            )

                    def emit_pool_chunk(j, s_acc=s_acc, res=res):
                        """pooled chunk j: needs S[512j-1 .. 512j+512]."""
                        lo = 512 * j - 1
                        hi = 512 * j + 513
                        sS = post_pool.tile([1, 516], FP32, tag="sS")
                        if lo < 0:
                            nc.vector.memset(sS[0:1, 0:1], 0.0)
                        if hi > L:
                            nc.vector.memset(sS[0:1, 513:514], 0.0)
                        src_lo = max(lo, 0)
                        dst_lo = src_lo - lo
                        src_hi = min(hi, L)
                        # tail chunks copy on ACT (idle there); early ones DVE
                        if j <= 1:
                            nc.scalar.copy(
                                out=sS[0:1, dst_lo : dst_lo + src_hi - src_lo],
                                in_=s_acc[0:1, src_lo:src_hi],
                            )
                        else:
                            nc.vector.tensor_copy(
                                out=sS[0:1, dst_lo : dst_lo + src_hi - src_lo],
                                in_=s_acc[0:1, src_lo:src_hi],
                            )
                        t1 = post_pool.tile([1, 512], FP32, tag="t1")
                        nc.vector.tensor_add(
                            out=t1, in0=sS[0:1, 0:512], in1=sS[0:1, 1:513]
                        )
                        t2 = post_pool.tile([1, 512], FP32, tag="t2")
                        nc.vector.tensor_add(out=t2, in0=t1, in1=sS[0:1, 2:514])
                        nc.vector.tensor_mul(
                            out=res[0:1, 512 * j : 512 * (j + 1)],
                            in0=t2,
                            in1=cntr[0:1, 512 * j : 512 * (j + 1)],
                        )

                    def maybe_pool(r_done, done_chunks=done_chunks,
                                   emit_pool_chunk=emit_pool_chunk):
                        # chunk c of S is final after block 15-4c; pooled
                        # chunk j additionally needs chunk j-1 (block 19-4j)
                        for j in range(3, -1, -1):
                            if j in done_chunks:
                                continue
                            need = 15 - 4 * (j - 1) if j >= 1 else 15
                            if r_done >= need:
                                done_chunks.add(j)
                                emit_pool_chunk(j)

                    for r in range(NBLK):
                        if b == 0 and prep1_late is not None and r == prep1_late:
                            QT[1], KT[1], WB[1] = emit_prep(1, weights, VR[1])
                        i0 = r * 128
                        ew = ew_pool.tile([128, W], FP16, tag="ew")
                        if opts["memset_dve"]:
                            nc.vector.memset(ew[:, L:W], 0.0)
                        else:
                            nc.gpsimd.memset(ew[:, L:W], 0.0)
                        dcol = []
                        for half in range(2):
                            c0 = half * 1024
                            ps = ps_pool.tile([128, 1024], FP32, tag="ps")
                            if opts["score_n1024"]:
                                nc.tensor.matmul(
                                    out=ps[:, :],
                                    lhsT=QT[b][:, i0 : i0 + 128],
                                    rhs=KT[b][:, c0 : c0 + 1024],
                                    start=True,
                                    stop=True,
                                )
                            else:
                                for j in range(2):
                                    nc.tensor.matmul(
                                        out=ps[:, j * 512 : (j + 1) * 512],
                                        lhsT=QT[b][:, i0 : i0 + 128],
                                        rhs=KT[b][:, c0 + j * 512 : c0 + (j + 1) * 512],
                                        start=True,
                                        stop=True,
                                    )
                            dc = small.tile([128, 1], FP32, tag="dc")
                            nc.scalar.activation(
                                out=ew[:, c0 : c0 + 1024],
                                in_=ps[:, :],
                                func=mybir.ActivationFunctionType.Exp,
                                scale=SCALE,
                                accum_out=dc,
                            )
                            dcol.append(dc)
                        dsum = small.tile([128, 1], FP32, tag="ds")
                        nc.vector.tensor_add(out=dsum, in0=dcol[0], in1=dcol[1])
                        drecip16 = small.tile([128, 1], FP16, tag="dr16")
                        with nc.allow_low_precision("1/d in fp16; error washes out"):
                            nc.vector.reciprocal(out=drecip16, in_=dsum)

                        # gate by w (columns i0..L of this block: upper triangle)
                        if not opts["skip_wmul"]:
                            if opts["wmul_split"]:
                                for c0_, c1_ in ((i0, 1024), (max(i0, 1024), L)):
                                    if c1_ > c0_:
                                        nc.vector.tensor_mul(
                                            out=ew[:, c0_:c1_],
                                            in0=ew[:, c0_:c1_],
                                            in1=WB[b][:, c0_:c1_],
                                        )
                            else:
                                nc.vector.tensor_mul(
                                    out=ew[:, i0:L],
                                    in0=ew[:, i0:L],
                                    in1=WB[b][:, i0:L],
                                )

                        # skewed write: scratch[i0+p, c - i0 - p] = ew[p, c]
                        # flat dst = i0*W + p*(W-1) + (c - i0)
                        wlen = (L + 127) - i0
                        dst = bass.AP(
                            tensor=scr[b],
                            offset=i0 * W,
                            ap=[[W - 1, 128], [1, wlen]],
                        )
                        wr = None
                        if not opts["skip_write"]:
                            weng = nc.gpsimd if opts["write_gpsimd"] else nc.sync
                            wr = weng.dma_start(out=dst, in_=ew[:, i0 : L + 127])
                            if (b, r) in prev_rd:
                                add_dep_helper(
                                    wr.ins, prev_rd[(b, r)], True,
                                    "scr WAR vs prev rep",
                                )

                        # plain read back: rows i0..i0+128, cols 0..L-i0
                        rlen = L - i0
                        src = bass.AP(
                            tensor=scr[b],
                            offset=i0 * W,
                            ap=[[W, 128], [1, rlen]],
                        )
                        rt = rt_pool.tile([128, L], FP16, tag="rt")
                        if not opts["skip_read"]:
                            rd = nc.sync.dma_start(out=rt[:, 0:rlen], in_=src)
                            if wr is not None:
                                add_dep_helper(rd.ins, wr.ins, True, "scratch RAW")
                            prev_rd[(b, r)] = rd.ins

                        # S[m] += sum_p (1/d[i0+p]) * rt[p, m] -- emitted a few
                        # blocks late so PE isn't head-of-line blocked on the
                        # scratch round-trip
                        if opts["skip_reduce"] or opts["skip_read"]:
                            continue
                        pending.append((r, rt, drecip16, rlen))
                        if len(pending) > REDUCE_DELAY:
                            item = pending.pop(0)
                            emit_reduce(item)
                            maybe_pool(item[0])

                    if not (opts["skip_reduce"] or opts["skip_read"]):
                        while pending:
                            item = pending.pop(0)
                            emit_reduce(item)
                            maybe_pool(item[0])

                    if opts["skip_post"] or opts["skip_reduce"] or opts["skip_read"]:
                        continue
                    nc.sync.dma_start(out=out[b, :], in_=res[0:1, :])

            for _rep in range(repeat):
                emit_instance()

    nc.finalize()
    return nc


_RUNNERS = {}


def _get_runner(repeat=1, **opts_kw):
    key = (repeat, tuple(sorted(opts_kw.items())))
    if key in _RUNNERS:
        return _RUNNERS[key]
    import jax
    from jax.experimental.shard_map import shard_map
    from jax.sharding import Mesh, PartitionSpec

    from concourse import bass2jax

    nc = build_nc(repeat, **opts_kw)
    bass2jax.install_neuronx_cc_hook()

    partition_name = nc.partition_id_tensor.name if nc.partition_id_tensor else None
    in_names, out_names, out_avals = [], [], []
    for alloc in nc.m.functions[0].allocations:
        if not isinstance(alloc, mybir.MemoryLocationSet):
            continue
        name = alloc.memorylocations[0].name
        if alloc.kind == "ExternalInput":
            if name != partition_name:
                in_names.append(name)
        elif alloc.kind == "ExternalOutput":
            out_names.append(name)
            out_avals.append(
                jax.core.ShapedArray(
                    tuple(alloc.tensor_shape), mybir.dt.np(alloc.dtype)
                )
            )
    n_params = len(in_names)
    n_outs = len(out_names)
    all_in = list(in_names) + list(out_names)
    if partition_name is not None:
        all_in.append(partition_name)

    def _body(*args):
        operands = list(args)
        if partition_name is not None:
            operands.append(bass2jax.partition_id_tensor())
        outs = bass2jax._bass_exec_p.bind(
            *operands,
            out_avals=tuple(out_avals),
            in_names=tuple(all_in),
            out_names=tuple(out_names),
            lowering_input_output_aliases=(),
            sim_require_finite=True,
            sim_require_nnan=True,
            nc=nc,
        )
        return tuple(outs)

    devices = jax.devices()[:H]
    mesh = Mesh(np.asarray(devices), ("core",))
    sharded = jax.jit(
        shard_map(
            _body,
            mesh=mesh,
            in_specs=(PartitionSpec("core"),) * (n_params + n_outs),
            out_specs=(PartitionSpec("core"),) * n_outs,
            check_rep=False,
        ),
        donate_argnums=tuple(range(n_params, n_params + n_outs)),
        keep_unused=True,
    )
    runner = (sharded, in_names, out_names, out_avals)
    _RUNNERS[key] = runner
    return runner


def _prep_in_maps(x, pe, Wq, bq, Wk, bk, Wv):
    x = np.asarray(x, np.float32)
    pe = np.asarray(pe, np.float32)
    Wq = np.asarray(Wq, np.float32)
    bq = np.asarray(bq, np.float32)
    Wk = np.asarray(Wk, np.float32)
    bk = np.asarray(bk, np.float32)
    Wv = np.asarray(Wv, np.float32)

    xp = x + pe[None, :, :]
    xpT = np.ascontiguousarray(xp.transpose(0, 2, 1)).astype(np.float16)
    xrevT = np.ascontiguousarray(x[:, ::-1, :].transpose(0, 2, 1)).astype(np.float16)

    in_maps = []
    for h in range(H):
        hs = slice(h * HD, (h + 1) * HD)
        bqk = np.concatenate([bq[hs], bk[hs]]).reshape(2 * HD, 1)
        in_maps.append(
            {
                "xpT": xpT,
                "xrevT": xrevT,
                "wq": np.ascontiguousarray(Wq[:, hs]).astype(np.float16),
                "wk": np.ascontiguousarray(Wk[:, hs]).astype(np.float16),
                "wv": np.ascontiguousarray(Wv[:, h : h + 1]).astype(np.float16),
                "bqk": np.ascontiguousarray(bqk).astype(np.float32),
            }
        )
    return in_maps


def run(in_maps, repeat=1, **opts_kw):
    sharded, in_names, out_names, out_avals = _get_runner(repeat, **opts_kw)
    concat_in = [
        np.concatenate([np.asarray(in_maps[c][n]) for c in range(H)], axis=0)
        for n in in_names
    ]
    concat_zeros = [
        np.zeros((H * a.shape[0], *a.shape[1:]), a.dtype) for a in out_avals
    ]
    out_arrs = sharded(*concat_in, *concat_zeros)
    return [
        {
            n: np.asarray(out_arrs[i]).reshape(H, *out_avals[i].shape)[c]
            for i, n in enumerate(out_names)
        }
        for c in range(H)
    ]


def kernel(x, pe, Wq, bq, Wk, bk, Wv):
    in_maps = _prep_in_maps(x, pe, Wq, bq, Wk, bk, Wv)
    zb = not (np.any(np.asarray(bq)) or np.any(np.asarray(bk)))
    results = run(in_maps, repeat=1, zero_bias=bool(zb))
    return np.stack([results[h]["out"] for h in range(H)], axis=2)



# revision 11
# speedup vs baseline: 1.1063x; 1.1063x over previous
"""Trainium2 Bass kernel for nn_MultiHeadDistanceLayer (sparse_attention).

Math: for each (head h, batch b) the reference collapses to
    S[m] = sum_k attn[k-m, k] * w[k],   w[k] = sigmoid(x @ Wv)[L-1-k, h]
(weighted superdiagonal sums of the attention matrix), followed by a
window-3 same-padded average pool over m (padding excluded from the
divisor):  out[b, m, h] = (S[m-1] + S[m] + S[m+1]) / cnt[m].

Sharding: 8 heads -> 8 NeuronCores; each core computes its head for both
batches.  Per (h, b) the kernel runs flash-style over 16 row blocks of 128
queries: scores via PE (K=HD=32), exp on ACT (with free row-sum accum ->
softmax denominators d), gating by w on DVE, then a *skewed* DMA write of
the probability block to a DRAM scratch so that superdiagonal m lands at
column m of every row.  A plain strided read back + a [1/(3d)]-weighted
ones-matmul on PE reduces partitions, PSUM-accumulating S/3 into a single
PSUM bank laid out as 4 rows at partitions {0,32,64,96} (512 cols each).
The window-3 pool then runs on DVE as 4-partition-parallel strided ops
(the /3 divisor is pre-folded; the two boundary elements get *1.5).
The sigmoid gate w is precomputed on the host (O(L*D) prep, like x+pe)
and arrives pre-broadcast as a [128, L] tile per batch, which removes the
sigmoid/exp ACT-table swap and the reversed-x input entirely.
"""

import contextlib

import numpy as np

import concourse.bacc as bacc
import concourse.bass as bass
import concourse.tile as tile
from concourse import mybir
from concourse.tile import add_dep_helper

B, L, D, H, HD, WIN = 2, 2048, 128, 8, 32, 3
NBLK = L // 128           # 16 row blocks per (h, b)
W = L + 128               # scratch row width (elements)
SCALE = float(HD) ** -0.5

FP16 = mybir.dt.float16
FP32 = mybir.dt.float32

DEFAULT_OPTS = dict(
    ew_bufs=10,
    rt_bufs=8,
    zero_bias=False,     # biases known to be zero -> plain copy instead of add
    kt_act=True,         # prep K-copies on ACT (idle during prep)
    prep1_late=2,        # emit pair-1 prep after this many pair-0 blocks
    reduce_delay=4,
    memset_pool=True,    # ew tail memset on Pool (gpsimd) instead of DVE
    small_bufs=4,
    score_n1024=False,   # single N=1024 scores matmul per psum tile
    read_vector=False,   # scratch read-back DMAs issued from DVE queue
    write_gpsimd=False,  # scratch writes via SWDGE (Pool) instead of HWDGE
    post_pool_copy=False,  # S psum->sbuf copy on Pool instead of DVE
)


def build_nc(repeat=1, **opts_kw):
    opts = dict(DEFAULT_OPTS, **opts_kw)
    nc = bacc.Bacc("TRN2", target_bir_lowering=False, debug=False)

    xpT = nc.dram_tensor("xpT", [B, D, L], FP16, kind="ExternalInput")
    wbrev = nc.dram_tensor("wbrev", [B, 128, L], FP16, kind="ExternalInput")
    wq = nc.dram_tensor("wq", [D, HD], FP16, kind="ExternalInput")
    wk = nc.dram_tensor("wk", [D, HD], FP16, kind="ExternalInput")
    bqk = nc.dram_tensor("bqk", [2 * HD, 1], FP32, kind="ExternalInput")
    out = nc.dram_tensor("out", [B, L], FP32, kind="ExternalOutput")
    # one flat scratch region per (h, b) pair; row i of the logical [L, W]
    # grid holds the skew-shifted probability row i
    scr = [
        nc.dram_tensor(f"scr{b}", [L * W], FP16, kind="Internal") for b in range(B)
    ]

    with tile.TileContext(nc) as tc:
        with contextlib.ExitStack() as ctx:
            singles = ctx.enter_context(tc.tile_pool(name="singles", bufs=1))
            small = ctx.enter_context(tc.tile_pool(name="small", bufs=opts["small_bufs"]))
            ew_pool = ctx.enter_context(tc.tile_pool(name="ew", bufs=opts["ew_bufs"]))
            rt_pool = ctx.enter_context(tc.tile_pool(name="rt", bufs=opts["rt_bufs"]))
            ps_pool = ctx.enter_context(tc.tile_pool(name="ps", bufs=2, space="PSUM"))
            s_pool = ctx.enter_context(tc.tile_pool(name="spsum", bufs=1, space="PSUM"))
            post_pool = ctx.enter_context(tc.tile_pool(name="post", bufs=2))

            # exp table warm-up: no data deps, so it schedules immediately and
            # the single act-table load lands before any real exp
            warm = singles.tile([1, 8], FP32)
            nc.vector.memset(warm, 0.0)
            warm2 = singles.tile([1, 8], FP32)
            nc.scalar.activation(out=warm2, in_=warm,
                                 func=mybir.ActivationFunctionType.Exp)
            zt = singles.tile([1, 2], FP32)
            nc.vector.memset(zt, 0.0)

            prev_rd = {}

            def emit_prep(b, weights):
                """QT/KT [32, L] fp16 and gate tile WB [128, L] fp16 for b."""
                bqk_sb, wq_sb, wk_sb = weights
                xp_t = singles.tile([D, L], FP16, tag=f"xpT{b}")
                for c0, c1 in ((0, 512), (512, 1024), (1024, L)):
                    nc.sync.dma_start(out=xp_t[:, c0:c1], in_=xpT[b, :, c0:c1])

                qt = singles.tile([HD, L], FP16, tag=f"QT{b}")
                kt = singles.tile([HD, L], FP16, tag=f"KT{b}")
                # packed prep psum: Q at partitions [0:32), K at [32:64);
                # copies emitted per 512-chunk so the psum slot frees early
                for half in range(2):
                    c0 = half * 1024
                    pqkv = ps_pool.tile([128, 1024], FP32, tag="ps")
                    for j in range(2):
                        cs = slice(c0 + j * 512, c0 + (j + 1) * 512)
                        js = slice(j * 512, (j + 1) * 512)
                        nc.tensor.matmul(
                            out=pqkv[0:HD, js], lhsT=wq_sb[:, :],
                            rhs=xp_t[:, cs], start=True, stop=True,
                        )
                        nc.tensor.matmul(
                            out=pqkv[HD : 2 * HD, js], lhsT=wk_sb[:, :],
                            rhs=xp_t[:, cs], start=True, stop=True,
                        )
                        if opts["zero_bias"]:
                            nc.vector.tensor_copy(
                                out=qt[:, cs], in_=pqkv[0:HD, js]
                            )
                            if opts["kt_act"]:
                                nc.scalar.copy(
                                    out=kt[:, cs], in_=pqkv[HD : 2 * HD, js]
                                )
                            else:
                                nc.vector.tensor_copy(
                                    out=kt[:, cs], in_=pqkv[HD : 2 * HD, js]
                                )
                        else:
                            nc.vector.tensor_scalar_add(
                                out=qt[:, cs],
                                in0=pqkv[0:HD, js],
                                scalar1=bqk_sb[0:HD],
                            )
                            if opts["kt_act"]:
                                nc.scalar.add(
                                    out=kt[:, cs],
                                    in_=pqkv[HD : 2 * HD, js],
                                    add=bqk_sb[HD : 2 * HD],
                                )
                            else:
                                nc.vector.tensor_scalar_add(
                                    out=kt[:, cs],
                                    in0=pqkv[HD : 2 * HD, js],
                                    scalar1=bqk_sb[HD : 2 * HD],
                                )
                # gate tile arrives pre-broadcast from the host
                wb = singles.tile([128, L], FP16, tag=f"WB{b}")
                for half in range(2):
                    c0 = half * 1024
                    nc.sync.dma_start(
                        out=wb[:, c0 : c0 + 1024], in_=wbrev[b, :, c0 : c0 + 1024]
                    )
                return qt, kt, wb

            def emit_post(b, st):
                """Window-3 pool of S'=S/3 + store.

                `st` is the SBUF copy of S' (rows at partitions 0/32/64/96,
                copied per segment as it finalized).  SBUF->SBUF DMAs gather
                it into a haloed tile Sc[4, 514] with
                Sc[j, t] = S'[512j + t - 1]; the pool is then two
                4-partition-parallel adds (engine ops need partition step 1
                and matching start partitions, so the cross-partition
                rearrangement must come via DMA).
                """
                sc = post_pool.tile([4, 514], FP32, tag="Sc")
                nc.sync.dma_start(out=sc[0:1, 0:1], in_=zt[0:1, 0:1])
                nc.sync.dma_start(out=sc[3:4, 513:514], in_=zt[0:1, 1:2])
                nc.sync.dma_start(out=sc[0:4, 1:513], in_=st[0:97:32, 0:512])
                nc.sync.dma_start(out=sc[1:4, 0:1], in_=st[0:65:32, 511:512])
                nc.sync.dma_start(out=sc[0:3, 513:514], in_=st[32:97:32, 0:1])
                u = post_pool.tile([4, 513], FP32, tag="u")
                nc.vector.tensor_add(
                    out=u[0:4, 0:513], in0=sc[0:4, 0:513], in1=sc[0:4, 1:514]
                )
                res = post_pool.tile([4, 512], FP32, tag="res")
                nc.vector.tensor_add(
                    out=res[0:4, 0:512], in0=u[0:4, 0:512], in1=sc[0:4, 2:514]
                )
                # the outermost two elements (cnt=2, x1.5) are fixed up on
                # the host: single-partition engine ops must start at a
                # quadrant partition, which row 3 col 511 cannot
                nc.sync.dma_start(out=out[b, :], in_=res[0:4, :])

            def emit_instance():
                bqk_sb = singles.tile([2 * HD, 1], FP32, tag="bqk_sb")
                if not opts["zero_bias"]:
                    nc.sync.dma_start(out=bqk_sb, in_=bqk[:, :])
                wq_sb = singles.tile([D, HD], FP16, tag="wq_sb")
                nc.sync.dma_start(out=wq_sb, in_=wq[:, :])
                wk_sb = singles.tile([D, HD], FP16, tag="wk_sb")
                nc.sync.dma_start(out=wk_sb, in_=wk[:, :])
                weights = (bqk_sb, wq_sb, wk_sb)

                QT, KT, WB = {}, {}, {}
                REDUCE_DELAY = opts["reduce_delay"]

                prep1_late = opts["prep1_late"]
                QT[0], KT[0], WB[0] = emit_prep(0, weights)
                if prep1_late is None:
                    QT[1], KT[1], WB[1] = emit_prep(1, weights)

                for b in range(B):
                    s4 = s_pool.tile([97, 512], FP32, tag="S4")
                    st = post_pool.tile([97, 512], FP32, tag="St")
                    pending = []

                    def emit_reduce(item, s4=s4, st=st):
                        rr, rt_t, dr16, rlen_r = item
                        for j in range(4):
                            m0 = j * 512
                            wj = min(512, rlen_r - m0)
                            if wj <= 0:
                                continue
                            nc.tensor.matmul(
                                out=s4[32 * j : 32 * j + 1, 0:wj],
                                lhsT=dr16[:, 0:1],
                                rhs=rt_t[:, m0 : m0 + wj],
                                start=(rr == 0),
                                stop=(rr == 15 - 4 * j),
                                tile_position=(0, 32 * j),
                            )
                            if rr == 15 - 4 * j:
                                # segment j closed: move it to SBUF now so
                                # the psum bank frees and the pool DMAs can
                                # read it (DMA cannot touch PSUM)
                                nc.vector.tensor_copy(
                                    out=st[32 * j : 32 * j + 1, :],
                                    in_=s4[32 * j : 32 * j + 1, :],
                                )

                    for r in range(NBLK):
                        if b == 0 and prep1_late is not None and r == prep1_late:
                            QT[1], KT[1], WB[1] = emit_prep(1, weights)
                        i0 = r * 128
                        ew = ew_pool.tile([128, W], FP16, tag="ew")
                        if opts["memset_pool"]:
                            nc.gpsimd.memset(ew[:, L:W], 0.0)
                        else:
                            nc.vector.memset(ew[:, L:W], 0.0)
                        dcol = []
                        for half in range(2):
                            c0 = half * 1024
                            ps = ps_pool.tile([128, 1024], FP32, tag="ps")
                            if opts["score_n1024"]:
                                nc.tensor.matmul(
                                    out=ps[:, :],
                                    lhsT=QT[b][:, i0 : i0 + 128],
                                    rhs=KT[b][:, c0 : c0 + 1024],
                                    start=True,
                                    stop=True,
                                )
                            else:
                                for j in range(2):
                                    nc.tensor.matmul(
                                        out=ps[:, j * 512 : (j + 1) * 512],
                                        lhsT=QT[b][:, i0 : i0 + 128],
                                        rhs=KT[b][:, c0 + j * 512 : c0 + (j + 1) * 512],
                                        start=True,
                                        stop=True,
                                    )
                            dc = small.tile([128, 1], FP32, tag="dc")
                            nc.scalar.activation(
                                out=ew[:, c0 : c0 + 1024],
                                in_=ps[:, :],
                                func=mybir.ActivationFunctionType.Exp,
                                scale=SCALE,
                                accum_out=dc,
                            )
                            dcol.append(dc)
                        dsum = small.tile([128, 1], FP32, tag="ds")
                        nc.vector.tensor_add(out=dsum, in0=dcol[0], in1=dcol[1])
                        dr32 = small.tile([128, 1], FP32, tag="dr32")
                        nc.vector.reciprocal_approx_fast(out=dr32, in_=dsum)
                        drecip16 = small.tile([128, 1], FP16, tag="dr16")
                        with nc.allow_low_precision("1/(3d) in fp16; washes out"):
                            nc.vector.tensor_scalar_mul(
                                out=drecip16, in0=dr32, scalar1=1.0 / 3.0
                            )

                        # gate by w (columns i0..L of this block: upper triangle)
                        nc.vector.tensor_mul(
                            out=ew[:, i0:L],
                            in0=ew[:, i0:L],
                            in1=WB[b][:, i0:L],
                        )

                        # skewed write: scratch[i0+p, c - i0 - p] = ew[p, c]
                        # flat dst = i0*W + p*(W-1) + (c - i0)
                        wlen = (L + 127) - i0
                        dst = bass.AP(
                            tensor=scr[b],
                            offset=i0 * W,
                            ap=[[W - 1, 128], [1, wlen]],
                        )
                        weng = nc.gpsimd if opts["write_gpsimd"] else nc.sync
                        wr = weng.dma_start(out=dst, in_=ew[:, i0 : L + 127])
                        if (b, r) in prev_rd:
                            add_dep_helper(
                                wr.ins, prev_rd[(b, r)], True,
                                "scr WAR vs prev rep",
                            )

                        # plain read back: rows i0..i0+128, cols 0..L-i0
                        rlen = L - i0
                        src = bass.AP(
                            tensor=scr[b],
                            offset=i0 * W,
                            ap=[[W, 128], [1, rlen]],
                        )
                        rt = rt_pool.tile([128, L], FP16, tag="rt")
                        reng = nc.vector if opts["read_vector"] else nc.sync
                        rd = reng.dma_start(out=rt[:, 0:rlen], in_=src)
                        add_dep_helper(rd.ins, wr.ins, True, "scratch RAW")
                        prev_rd[(b, r)] = rd.ins

                        # S'[m] += sum_p (1/(3 d[i0+p])) * rt[p, m] -- emitted a
                        # few blocks late so PE isn't head-of-line blocked on
                        # the scratch round-trip
                        pending.append((r, rt, drecip16, rlen))
                        if len(pending) > REDUCE_DELAY:
                            emit_reduce(pending.pop(0))

                    while pending:
                        emit_reduce(pending.pop(0))

                    emit_post(b, st)

            for _rep in range(repeat):
                emit_instance()

    nc.finalize()
    return nc


_RUNNERS = {}


def _get_runner(repeat=1, **opts_kw):
    key = (repeat, tuple(sorted(opts_kw.items())))
    if key in _RUNNERS:
        return _RUNNERS[key]
    import jax
    from jax.experimental.shard_map import shard_map
    from jax.sharding import Mesh, PartitionSpec

    from concourse import bass2jax

    nc = build_nc(repeat, **opts_kw)
    bass2jax.install_neuronx_cc_hook()

    partition_name = nc.partition_id_tensor.name if nc.partition_id_tensor else None
    in_names, out_names, out_avals = [], [], []
    for alloc in nc.m.functions[0].allocations:
        if not isinstance(alloc, mybir.MemoryLocationSet):
            continue
        name = alloc.memorylocations[0].name
        if alloc.kind == "ExternalInput":
            if name != partition_name:
                in_names.append(name)
        elif alloc.kind == "ExternalOutput":
            out_names.append(name)
            out_avals.append(
                jax.core.ShapedArray(
                    tuple(alloc.tensor_shape), mybir.dt.np(alloc.dtype)
                )
            )
    n_params = len(in_names)
    n_outs = len(out_names)
    all_in = list(in_names) + list(out_names)
    if partition_name is not None:
        all_in.append(partition_name)

    def _body(*args):
        operands = list(args)
        if partition_name is not None:
            operands.append(bass2jax.partition_id_tensor())
        outs = bass2jax._bass_exec_p.bind(
            *operands,
            out_avals=tuple(out_avals),
            in_names=tuple(all_in),
            out_names=tuple(out_names),
            lowering_input_output_aliases=(),
            sim_require_finite=True,
            sim_require_nnan=True,
            nc=nc,
        )
        return tuple(outs)

    devices = jax.devices()[:H]
    mesh = Mesh(np.asarray(devices), ("core",))
    sharded = jax.jit(
        shard_map(
            _body,
            mesh=mesh,
            in_specs=(PartitionSpec("core"),) * (n_params + n_outs),
            out_specs=(PartitionSpec("core"),) * n_outs,
            check_rep=False,
        ),
        donate_argnums=tuple(range(n_params, n_params + n_outs)),
        keep_unused=True,
    )
    runner = (sharded, in_names, out_names, out_avals)
    _RUNNERS[key] = runner
    return runner


def _prep_in_maps(x, pe, Wq, bq, Wk, bk, Wv):
    x = np.asarray(x, np.float32)
    pe = np.asarray(pe, np.float32)
    Wq = np.asarray(Wq, np.float32)
    bq = np.asarray(bq, np.float32)
    Wk = np.asarray(Wk, np.float32)
    bk = np.asarray(bk, np.float32)
    Wv = np.asarray(Wv, np.float32)

    xp = x + pe[None, :, :]
    xpT = np.ascontiguousarray(xp.transpose(0, 2, 1)).astype(np.float16)
    # gate w[k] = sigmoid(x @ Wv)[L-1-k, h], broadcast to 128 partitions
    vlog = np.einsum("bld,dh->blh", x.astype(np.float64), Wv.astype(np.float64))
    vsig = 1.0 / (1.0 + np.exp(-vlog))          # (B, L, H)
    vrev = vsig[:, ::-1, :]                      # (B, L, H) reversed over l

    in_maps = []
    for h in range(H):
        hs = slice(h * HD, (h + 1) * HD)
        bqk = np.concatenate([bq[hs], bk[hs]]).reshape(2 * HD, 1)
        wb = np.ascontiguousarray(
            np.broadcast_to(vrev[:, None, :, h], (B, 128, L))
        ).astype(np.float16)
        in_maps.append(
            {
                "xpT": xpT,
                "wbrev": wb,
                "wq": np.ascontiguousarray(Wq[:, hs]).astype(np.float16),
                "wk": np.ascontiguousarray(Wk[:, hs]).astype(np.float16),
                "bqk": np.ascontiguousarray(bqk).astype(np.float32),
            }
        )
    return in_maps


def run(in_maps, repeat=1, **opts_kw):
    sharded, in_names, out_names, out_avals = _get_runner(repeat, **opts_kw)
    concat_in = [
        np.concatenate([np.asarray(in_maps[c][n]) for c in range(H)], axis=0)
        for n in in_names
    ]
    concat_zeros = [
        np.zeros((H * a.shape[0], *a.shape[1:]), a.dtype) for a in out_avals
    ]
    out_arrs = sharded(*concat_in, *concat_zeros)
    return [
        {
            n: np.asarray(out_arrs[i]).reshape(H, *out_avals[i].shape)[c]
            for i, n in enumerate(out_names)
        }
        for c in range(H)
    ]


def kernel(x, pe, Wq, bq, Wk, bk, Wv):
    in_maps = _prep_in_maps(x, pe, Wq, bq, Wk, bk, Wv)
    zb = not (np.any(np.asarray(bq)) or np.any(np.asarray(bk)))
    results = run(in_maps, repeat=1, zero_bias=bool(zb))
    res = np.stack([results[h]["out"] for h in range(H)], axis=2)
    # window-3 SAME avg-pool divisor: ends divide by 2, not 3 (the kernel
    # pre-folds 1/3 into the reduce weights, so the two edge columns need
    # a 3/2 correction)
    res[:, 0, :] *= 1.5
    res[:, L - 1, :] *= 1.5
    return res


# revision 22
# speedup vs baseline: 1.1191x; 1.0116x over previous
"""Trainium2 Bass kernel for nn_MultiHeadDistanceLayer (sparse_attention).

Math: for each (head h, batch b) the reference collapses to
    S[m] = sum_k attn[k-m, k] * w[k],   w[k] = sigmoid(x @ Wv)[L-1-k, h]
(weighted superdiagonal sums of the attention matrix), followed by a
window-3 same-padded average pool over m (padding excluded from the
divisor):  out[b, m, h] = (S[m-1] + S[m] + S[m+1]) / cnt[m].

Sharding: 8 heads -> 8 NeuronCores; each core computes its head for both
batches.  Per (h, b) the kernel runs flash-style over 16 row blocks of 128
queries: scores via PE (K=HD=32), exp on ACT (with free row-sum accum ->
softmax denominators d), gating by w on DVE, then a *skewed* DMA write of
the probability block to a DRAM scratch so that superdiagonal m lands at
column m of every row.  A plain strided read back + a [1/(3d)]-weighted
ones-matmul on PE reduces partitions, PSUM-accumulating S/3 into a single
PSUM bank laid out as 4 rows at partitions {0,32,64,96} (512 cols each).
The window-3 pool then runs on DVE as 4-partition-parallel strided ops
(the /3 divisor is pre-folded; the two boundary elements get *1.5).
The sigmoid gate w is precomputed on the host (O(L*D) prep, like x+pe)
and arrives pre-broadcast as a [128, L] tile per batch, which removes the
sigmoid/exp ACT-table swap and the reversed-x input entirely.
"""

import contextlib

import numpy as np

import concourse.bacc as bacc
import concourse.bass as bass
import concourse.tile as tile
from concourse import mybir
from concourse.tile import add_dep_helper

B, L, D, H, HD, WIN = 2, 2048, 128, 8, 32, 3
NBLK = L // 128           # 16 row blocks per (h, b)
W = L + 128               # scratch row width (elements)
SCALE = float(HD) ** -0.5

FP16 = mybir.dt.float16
FP32 = mybir.dt.float32

DEFAULT_OPTS = dict(
    ew_bufs=10,
    rt_bufs=8,
    zero_bias=False,     # biases known to be zero -> plain copy instead of add
    kt_act=True,         # prep K-copies on ACT (idle during prep)
    prep1_late=2,        # emit pair-1 prep after this many pair-0 blocks
    reduce_delay=4,
    memset_pool=True,    # ew tail memset on Pool (gpsimd) instead of DVE
    small_bufs=4,
    score_n1024=False,   # single N=1024 scores matmul per psum tile
    read_vector=False,   # scratch read-back DMAs issued from DVE queue
    write_gpsimd=False,  # scratch writes via SWDGE (Pool) instead of HWDGE
    post_pool_copy=False,  # S psum->sbuf copy on Pool instead of DVE
)


def build_nc(repeat=1, **opts_kw):
    opts = dict(DEFAULT_OPTS, **opts_kw)
    nc = bacc.Bacc("TRN2", target_bir_lowering=False, debug=False)

    xpT = nc.dram_tensor("xpT", [B, D, L], FP16, kind="ExternalInput")
    wbrev = nc.dram_tensor("wbrev", [B, 128, L], FP16, kind="ExternalInput")
    wqk = nc.dram_tensor("wqk", [D, 2 * HD], FP16, kind="ExternalInput")
    bqk = nc.dram_tensor("bqk", [2 * HD, 1], FP32, kind="ExternalInput")
    out = nc.dram_tensor("out", [B, L], FP32, kind="ExternalOutput")
    # one flat scratch region per (h, b) pair; row i of the logical [L, W]
    # grid holds the skew-shifted probability row i
    scr = [
        nc.dram_tensor(f"scr{b}", [L * W], FP16, kind="Internal") for b in range(B)
    ]

    with tile.TileContext(nc) as tc:
        with contextlib.ExitStack() as ctx:
            singles = ctx.enter_context(tc.tile_pool(name="singles", bufs=1))
            small = ctx.enter_context(tc.tile_pool(name="small", bufs=opts["small_bufs"]))
            ew_pool = ctx.enter_context(tc.tile_pool(name="ew", bufs=opts["ew_bufs"]))
            rt_pool = ctx.enter_context(tc.tile_pool(name="rt", bufs=opts["rt_bufs"]))
            ps_pool = ctx.enter_context(tc.tile_pool(name="ps", bufs=2, space="PSUM"))
            prep_pool = ctx.enter_context(tc.tile_pool(name="prep", bufs=1, space="PSUM"))
            s_pool = ctx.enter_context(tc.tile_pool(name="spsum", bufs=1, space="PSUM"))
            post_pool = ctx.enter_context(tc.tile_pool(name="post", bufs=2))

            # exp table warm-up: no data deps, so it schedules immediately and
            # the single act-table load lands before any real exp
            warm = singles.tile([1, 8], FP32)
            nc.vector.memset(warm, 0.0)
            warm2 = singles.tile([1, 8], FP32)
            nc.scalar.activation(out=warm2, in_=warm,
                                 func=mybir.ActivationFunctionType.Exp)

            prev_rd = {}

            def emit_prep(b, weights):
                """QT/KT [32, L] fp16 and gate tile WB [128, L] fp16 for b."""
                bqk_sb, wqk_sb = weights
                wq_sb = wqk_sb[:, 0:HD]
                wk_sb = wqk_sb[:, HD : 2 * HD]
                xp_t = singles.tile([D, L], FP16, tag=f"xpT{b}")
                for c0, c1 in ((0, 1024), (1024, L)):
                    nc.sync.dma_start(out=xp_t[:, c0:c1], in_=xpT[b, :, c0:c1])

                qt = singles.tile([HD, L], FP16, tag=f"QT{b}")
                kt = singles.tile([HD, L], FP16, tag=f"KT{b}")
                # packed prep psum: Q at partitions [0:32), K at [32:64);
                # copies emitted per 512-chunk so the psum slot frees early
                for half in range(2):
                    c0 = half * 1024
                    pqkv = prep_pool.tile([128, 1024], FP32, tag="prep")
                    for j in range(2):
                        cs = slice(c0 + j * 512, c0 + (j + 1) * 512)
                        js = slice(j * 512, (j + 1) * 512)
                        nc.tensor.matmul(
                            out=pqkv[0:HD, js], lhsT=wq_sb[:, :],
                            rhs=xp_t[:, cs], start=True, stop=True,
                        )
                        nc.tensor.matmul(
                            out=pqkv[HD : 2 * HD, js], lhsT=wk_sb[:, :],
                            rhs=xp_t[:, cs], start=True, stop=True,
                        )
                        if opts["zero_bias"]:
                            nc.vector.tensor_copy(
                                out=qt[:, cs], in_=pqkv[0:HD, js]
                            )
                            if opts["kt_act"]:
                                nc.scalar.copy(
                                    out=kt[:, cs], in_=pqkv[HD : 2 * HD, js]
                                )
                            else:
                                nc.vector.tensor_copy(
                                    out=kt[:, cs], in_=pqkv[HD : 2 * HD, js]
                                )
                        else:
                            nc.vector.tensor_scalar_add(
                                out=qt[:, cs],
                                in0=pqkv[0:HD, js],
                                scalar1=bqk_sb[0:HD],
                            )
                            if opts["kt_act"]:
                                nc.scalar.add(
                                    out=kt[:, cs],
                                    in_=pqkv[HD : 2 * HD, js],
                                    add=bqk_sb[HD : 2 * HD],
                                )
                            else:
                                nc.vector.tensor_scalar_add(
                                    out=kt[:, cs],
                                    in0=pqkv[HD : 2 * HD, js],
                                    scalar1=bqk_sb[HD : 2 * HD],
                                )
                # gate tile arrives pre-broadcast from the host
                wb = singles.tile([128, L], FP16, tag=f"WB{b}")
                for half in range(2):
                    c0 = half * 1024
                    nc.sync.dma_start(
                        out=wb[:, c0 : c0 + 1024], in_=wbrev[b, :, c0 : c0 + 1024]
                    )
                return qt, kt, wb

            def emit_post_row(b, j, st, u, res):
                """Pool one 512-col segment (row at partition 32j).

                st[32j, t] = S'[512j + t - 1] (haloed; t in [0, 514)), so
                    u[t]   = st[t] + st[t+1]
                    res[c] = u[c] + st[c+2]  -> pooled[512j + c]
                All ops stay on partition 32j (engine ops need matching
                quadrant start partitions).
                """
                p = slice(32 * j, 32 * j + 1)
                nc.vector.tensor_add(
                    out=u[p, 0:513], in0=st[p, 0:513], in1=st[p, 1:514]
                )
                nc.vector.tensor_add(
                    out=res[p, 0:512], in0=u[p, 0:512], in1=st[p, 2:514]
                )

            def emit_instance():
                bqk_sb = singles.tile([2 * HD, 1], FP32, tag="bqk_sb")
                if not opts["zero_bias"]:
                    nc.sync.dma_start(out=bqk_sb, in_=bqk[:, :])
                wqk_sb = singles.tile([D, 2 * HD], FP16, tag="wqk_sb")
                nc.sync.dma_start(out=wqk_sb, in_=wqk[:, :])
                weights = (bqk_sb, wqk_sb)

                QT, KT, WB = {}, {}, {}
                REDUCE_DELAY = opts["reduce_delay"]

                prep1_late = opts["prep1_late"]
                QT[0], KT[0], WB[0] = emit_prep(0, weights)
                if prep1_late is None:
                    QT[1], KT[1], WB[1] = emit_prep(1, weights)

                for b in range(B):
                    # haloed psum accumulator: row 32j col t holds
                    # S'[512j + t - 1] for t in [0, 514); the never-written
                    # edge cols (row0 t=0, row3 t=513) are zeroed up front
                    s4 = s_pool.tile([97, 516], FP32, tag="S4")
                    nc.vector.memset(s4[0:1, 0:1], 0.0)
                    nc.vector.memset(s4[96:97, 513:514], 0.0)
                    st = post_pool.tile([97, 514], FP32, tag="St")
                    u4 = post_pool.tile([97, 513], FP32, tag="u4")
                    res = post_pool.tile([97, 512], FP32, tag="res")
                    # row j's psum group closes one block after segment j's
                    # data cols (the 1-col left-halo tail), except row 0
                    rmax = [15, 12, 8, 4]
                    pending = []

                    def emit_reduce(item, s4=s4, st=st, u4=u4, res=res,
                                    rmax=rmax, b=b):
                        rr, rt_t, dr16, rlen_r = item
                        for j in range(4):
                            g0 = 512 * j - 1      # global col of local t=0
                            glo = max(0, g0)
                            ghi = min(rlen_r, 512 * j + 513)
                            if ghi <= glo:
                                continue
                            t0 = glo - g0
                            t1 = ghi - g0
                            # moving free dim caps at 512 -> chunk the window;
                            # each chunk's psum group stops at the last block
                            # whose read still covers the chunk's first col
                            for c0_ in range(t0, t1, 512):
                                c1_ = min(c0_ + 512, t1)
                                nc.tensor.matmul(
                                    out=s4[32 * j : 32 * j + 1, c0_:c1_],
                                    lhsT=dr16[:, 0:1],
                                    rhs=rt_t[:, g0 + c0_ : g0 + c1_],
                                    start=(rr == 0),
                                    stop=(rr == (2047 - (g0 + c0_)) // 128),
                                    tile_position=(0, 32 * j),
                                )
                            if rr == rmax[j]:
                                # row j closed: move to SBUF (frees nothing
                                # yet, but hides the copy + pool off the
                                # tail for rows 1..3)
                                p = slice(32 * j, 32 * j + 1)
                                nc.vector.tensor_copy(
                                    out=st[p, :], in_=s4[p, 0:514]
                                )
                                emit_post_row(b, j, st, u4, res)

                    for r in range(NBLK):
                        if b == 0 and prep1_late is not None and r == prep1_late:
                            QT[1], KT[1], WB[1] = emit_prep(1, weights)
                        i0 = r * 128
                        ew = ew_pool.tile([128, W], FP16, tag="ew")
                        if opts["memset_pool"]:
                            nc.gpsimd.memset(ew[:, L:W], 0.0)
                        else:
                            nc.vector.memset(ew[:, L:W], 0.0)
                        dcol = []
                        for half in range(2):
                            c0 = half * 1024
                            ps = ps_pool.tile([128, 1024], FP32, tag="ps")
                            if opts["score_n1024"]:
                                nc.tensor.matmul(
                                    out=ps[:, :],
                                    lhsT=QT[b][:, i0 : i0 + 128],
                                    rhs=KT[b][:, c0 : c0 + 1024],
                                    start=True,
                                    stop=True,
                                )
                            else:
                                for j in range(2):
                                    nc.tensor.matmul(
                                        out=ps[:, j * 512 : (j + 1) * 512],
                                        lhsT=QT[b][:, i0 : i0 + 128],
                                        rhs=KT[b][:, c0 + j * 512 : c0 + (j + 1) * 512],
                                        start=True,
                                        stop=True,
                                    )
                            dc = small.tile([128, 1], FP32, tag="dc")
                            nc.scalar.activation(
                                out=ew[:, c0 : c0 + 1024],
                                in_=ps[:, :],
                                func=mybir.ActivationFunctionType.Exp,
                                scale=SCALE,
                                accum_out=dc,
                            )
                            dcol.append(dc)
                        dsum = small.tile([128, 1], FP32, tag="ds")
                        nc.vector.tensor_add(out=dsum, in0=dcol[0], in1=dcol[1])
                        dr32 = small.tile([128, 1], FP32, tag="dr32")
                        nc.vector.reciprocal_approx_fast(out=dr32, in_=dsum)
                        drecip16 = small.tile([128, 1], FP16, tag="dr16")
                        with nc.allow_low_precision("1/(3d) in fp16; washes out"):
                            nc.vector.tensor_scalar_mul(
                                out=drecip16, in0=dr32, scalar1=1.0 / 3.0
                            )

                        # gate by w (columns i0..L of this block: upper triangle)
                        nc.vector.tensor_mul(
                            out=ew[:, i0:L],
                            in0=ew[:, i0:L],
                            in1=WB[b][:, i0:L],
                        )

                        # skewed write: scratch[i0+p, c - i0 - p] = ew[p, c]
                        # flat dst = i0*W + p*(W-1) + (c - i0)
                        wlen = (L + 127) - i0
                        dst = bass.AP(
                            tensor=scr[b],
                            offset=i0 * W,
                            ap=[[W - 1, 128], [1, wlen]],
                        )
                        weng = nc.gpsimd if opts["write_gpsimd"] else nc.sync
                        wr = weng.dma_start(out=dst, in_=ew[:, i0 : L + 127])
                        if (b, r) in prev_rd:
                            add_dep_helper(
                                wr.ins, prev_rd[(b, r)], True,
                                "scr WAR vs prev rep",
                            )

                        # plain read back: rows i0..i0+128, cols 0..L-i0
                        rlen = L - i0
                        src = bass.AP(
                            tensor=scr[b],
                            offset=i0 * W,
                            ap=[[W, 128], [1, rlen]],
                        )
                        rt = rt_pool.tile([128, L], FP16, tag="rt")
                        reng = nc.vector if opts["read_vector"] else nc.sync
                        rd = reng.dma_start(out=rt[:, 0:rlen], in_=src)
                        add_dep_helper(rd.ins, wr.ins, True, "scratch RAW")
                        prev_rd[(b, r)] = rd.ins

                        # S'[m] += sum_p (1/(3 d[i0+p])) * rt[p, m] -- emitted a
                        # few blocks late so PE isn't head-of-line blocked on
                        # the scratch round-trip
                        pending.append((r, rt, drecip16, rlen))
                        if len(pending) > REDUCE_DELAY:
                            emit_reduce(pending.pop(0))

                    while pending:
                        emit_reduce(pending.pop(0))

                    nc.sync.dma_start(out=out[b, :], in_=res[0:97:32, :])

            for _rep in range(repeat):
                emit_instance()

    nc.finalize()
    return nc


_RUNNERS = {}


def _get_runner(repeat=1, **opts_kw):
    key = (repeat, tuple(sorted(opts_kw.items())))
    if key in _RUNNERS:
        return _RUNNERS[key]
    import jax
    from jax.experimental.shard_map import shard_map
    from jax.sharding import Mesh, PartitionSpec

    from concourse import bass2jax

    nc = build_nc(repeat, **opts_kw)
    bass2jax.install_neuronx_cc_hook()

    partition_name = nc.partition_id_tensor.name if nc.partition_id_tensor else None
    in_names, out_names, out_avals = [], [], []
    for alloc in nc.m.functions[0].allocations:
        if not isinstance(alloc, mybir.MemoryLocationSet):
            continue
        name = alloc.memorylocations[0].name
        if alloc.kind == "ExternalInput":
            if name != partition_name:
                in_names.append(name)
        elif alloc.kind == "ExternalOutput":
            out_names.append(name)
            out_avals.append(
                jax.core.ShapedArray(
                    tuple(alloc.tensor_shape), mybir.dt.np(alloc.dtype)
                )
            )
    n_params = len(in_names)
    n_outs = len(out_names)
    all_in = list(in_names) + list(out_names)
    if partition_name is not None:
        all_in.append(partition_name)

    def _body(*args):
        operands = list(args)
        if partition_name is not None:
            operands.append(bass2jax.partition_id_tensor())
        outs = bass2jax._bass_exec_p.bind(
            *operands,
            out_avals=tuple(out_avals),
            in_names=tuple(all_in),
            out_names=tuple(out_names),
            lowering_input_output_aliases=(),
            sim_require_finite=True,
            sim_require_nnan=True,
            nc=nc,
        )
        return tuple(outs)

    devices = jax.devices()[:H]
    mesh = Mesh(np.asarray(devices), ("core",))
    sharded = jax.jit(
        shard_map(
            _body,
            mesh=mesh,
            in_specs=(PartitionSpec("core"),) * (n_params + n_outs),
            out_specs=(PartitionSpec("core"),) * n_outs,
            check_rep=False,
        ),
        donate_argnums=tuple(range(n_params, n_params + n_outs)),
        keep_unused=True,
    )
    runner = (sharded, in_names, out_names, out_avals)
    _RUNNERS[key] = runner
    return runner


def _prep_in_maps(x, pe, Wq, bq, Wk, bk, Wv):
    x = np.asarray(x, np.float32)
    pe = np.asarray(pe, np.float32)
    Wq = np.asarray(Wq, np.float32)
    bq = np.asarray(bq, np.float32)
    Wk = np.asarray(Wk, np.float32)
    bk = np.asarray(bk, np.float32)
    Wv = np.asarray(Wv, np.float32)

    xp = x + pe[None, :, :]
    xpT = np.ascontiguousarray(xp.transpose(0, 2, 1)).astype(np.float16)
    # gate w[k] = sigmoid(x @ Wv)[L-1-k, h], broadcast to 128 partitions
    vlog = np.einsum("bld,dh->blh", x.astype(np.float64), Wv.astype(np.float64))
    vsig = 1.0 / (1.0 + np.exp(-vlog))          # (B, L, H)
    vrev = vsig[:, ::-1, :]                      # (B, L, H) reversed over l

    in_maps = []
    for h in range(H):
        hs = slice(h * HD, (h + 1) * HD)
        bqk = np.concatenate([bq[hs], bk[hs]]).reshape(2 * HD, 1)
        wb = np.ascontiguousarray(
            np.broadcast_to(vrev[:, None, :, h], (B, 128, L))
        ).astype(np.float16)
        wqk = np.concatenate([Wq[:, hs], Wk[:, hs]], axis=1)
        in_maps.append(
            {
                "xpT": xpT,
                "wbrev": wb,
                "wqk": np.ascontiguousarray(wqk).astype(np.float16),
                "bqk": np.ascontiguousarray(bqk).astype(np.float32),
            }
        )
    return in_maps


def run(in_maps, repeat=1, **opts_kw):
    sharded, in_names, out_names, out_avals = _get_runner(repeat, **opts_kw)
    concat_in = [
        np.concatenate([np.asarray(in_maps[c][n]) for c in range(H)], axis=0)
        for n in in_names
    ]
    concat_zeros = [
        np.zeros((H * a.shape[0], *a.shape[1:]), a.dtype) for a in out_avals
    ]
    out_arrs = sharded(*concat_in, *concat_zeros)
    return [
        {
            n: np.asarray(out_arrs[i]).reshape(H, *out_avals[i].shape)[c]
            for i, n in enumerate(out_names)
        }
        for c in range(H)
    ]


def kernel(x, pe, Wq, bq, Wk, bk, Wv):
    in_maps = _prep_in_maps(x, pe, Wq, bq, Wk, bk, Wv)
    zb = not (np.any(np.asarray(bq)) or np.any(np.asarray(bk)))
    results = run(in_maps, repeat=1, zero_bias=bool(zb))
    res = np.stack([results[h]["out"] for h in range(H)], axis=2)
    # window-3 SAME avg-pool divisor: ends divide by 2, not 3 (the kernel
    # pre-folds 1/3 into the reduce weights, so the two edge columns need
    # a 3/2 correction)
    res[:, 0, :] *= 1.5
    res[:, L - 1, :] *= 1.5
    return res


# revision 39
# speedup vs baseline: 1.2232x; 1.0931x over previous
"""Trainium2 Bass kernel for nn_MultiHeadDistanceLayer (sparse_attention).

Math: for each (head h, batch b) the reference collapses to
    S[m] = sum_k attn[k-m, k] * w[k],   w[k] = sigmoid(x @ Wv)[L-1-k, h]
(weighted superdiagonal sums of the attention matrix), followed by a
window-3 same-padded average pool over m (padding excluded from the
divisor):  out[b, m, h] = (S[m-1] + S[m] + S[m+1]) / cnt[m].

Sharding: 8 heads -> 8 NeuronCores; each core computes its head for both
batches.  Per (h, b) the kernel runs flash-style over 16 row blocks of 128
queries: scores via PE (K=HD=32), exp on ACT (with free row-sum accum ->
softmax denominators d), gating by w on DVE, then a *skewed* DMA write of
the probability block to a DRAM scratch so that superdiagonal m lands at
column m of every row.  A plain strided read back + a [1/(3d)]-weighted
ones-matmul on PE reduces partitions, PSUM-accumulating S/3 into a single
PSUM bank laid out as 4 rows at partitions {0,32,64,96} (512 cols each).
The window-3 pool then runs on DVE as 4-partition-parallel strided ops
(the /3 divisor is pre-folded; the two boundary elements get *1.5).
The sigmoid gate w is precomputed on the host (O(L*D) prep, like x+pe)
and arrives pre-broadcast as a [128, L] tile per batch, which removes the
sigmoid/exp ACT-table swap and the reversed-x input entirely.
"""

import contextlib

import numpy as np

import concourse.bacc as bacc
import concourse.bass as bass
import concourse.tile as tile
from concourse import mybir
from concourse.tile import add_dep_helper

B, L, D, H, HD, WIN = 2, 2048, 128, 8, 32, 3
NBLK = L // 128           # 16 row blocks per (h, b)
W = L + 128               # scratch row width (elements)
SCALE = float(HD) ** -0.5

FP16 = mybir.dt.float16
FP32 = mybir.dt.float32

DEFAULT_OPTS = dict(
    ew_bufs=10,
    rt_bufs=10,
    zero_bias=False,     # biases known to be zero -> plain copy instead of add
    kt_act=False,        # prep K-copies on DVE (ACT stays exp-only)
    reduce_delay=6,
    memset_pool=False,   # ew tail memset on DVE
    small_bufs=8,
    read_act=False,      # scratch read-back DMAs issued from ACT queue
    write_gpsimd=False,  # scratch writes via SWDGE (Pool) instead of HWDGE
)


def build_nc(repeat=1, **opts_kw):
    opts = dict(DEFAULT_OPTS, **opts_kw)
    nc = bacc.Bacc("TRN2", target_bir_lowering=False, debug=False)

    xpT = nc.dram_tensor("xpT", [B, D, L], FP16, kind="ExternalInput")
    wbrev = nc.dram_tensor("wbrev", [B, 128, L], FP16, kind="ExternalInput")
    wqk = nc.dram_tensor("wqk", [D, 2 * HD], FP16, kind="ExternalInput")
    bqk = nc.dram_tensor("bqk", [2 * HD, 1], FP32, kind="ExternalInput")
    out = nc.dram_tensor("out", [B, L], FP32, kind="ExternalOutput")
    # one flat scratch region per (h, b) pair; row i of the logical [L, W]
    # grid holds the skew-shifted probability row i
    scr = [
        nc.dram_tensor(f"scr{b}", [L * W], FP16, kind="Internal") for b in range(B)
    ]

    with tile.TileContext(nc) as tc:
        with contextlib.ExitStack() as ctx:
            singles = ctx.enter_context(tc.tile_pool(name="singles", bufs=1))
            small = ctx.enter_context(tc.tile_pool(name="small", bufs=opts["small_bufs"]))
            ew_pool = ctx.enter_context(tc.tile_pool(name="ew", bufs=opts["ew_bufs"]))
            rt_pool = ctx.enter_context(tc.tile_pool(name="rt", bufs=opts["rt_bufs"]))
            ps_pool = ctx.enter_context(tc.tile_pool(name="ps", bufs=2, space="PSUM"))
            prep_pool = ctx.enter_context(tc.tile_pool(name="prep", bufs=2, space="PSUM"))
            s_pool = ctx.enter_context(tc.tile_pool(name="spsum", bufs=1, space="PSUM"))
            post_pool = ctx.enter_context(tc.tile_pool(name="post", bufs=2))

            # exp table warm-up: no data deps, so it schedules immediately and
            # the single act-table load lands before any real exp
            warm = singles.tile([1, 8], FP32)
            nc.vector.memset(warm, 0.0)
            warm2 = singles.tile([1, 8], FP32)
            nc.scalar.activation(out=warm2, in_=warm,
                                 func=mybir.ActivationFunctionType.Exp)

            prev_rd = {}

            def emit_prep(b, weights):
                """QT/KT [32, L] fp16 and gate tile WB [128, L] fp16 for b.

                `slot` maps a global 512-col psum chunk index into the shared
                6-slot score region.  K is computed/copied first per chunk:
                the first score matmul needs both KT chunks of half 0 but
                only the first QT chunk, so K is the critical path.
                """
                bqk_sb, wqk_sb = weights
                wq_sb = wqk_sb[:, 0:HD]
                wk_sb = wqk_sb[:, HD : 2 * HD]
                xp_t = singles.tile([D, L], FP16, tag=f"xpT{b}")
                for c0, c1 in ((0, 1024), (1024, L)):
                    nc.sync.dma_start(out=xp_t[:, c0:c1], in_=xpT[b, :, c0:c1])

                qt = singles.tile([HD, L], FP16, tag=f"QT{b}")
                kt = singles.tile([HD, L], FP16, tag=f"KT{b}")
                # packed prep psum: Q at partitions [0:32), K at [32:64)
                for ch in range(4):
                    cs = slice(ch * 512, (ch + 1) * 512)
                    pk = prep_pool.tile([128, 512], FP32, tag="prep")
                    nc.tensor.matmul(
                        out=pk[HD : 2 * HD, :], lhsT=wk_sb[:, :],
                        rhs=xp_t[:, cs], start=True, stop=True,
                    )
                    if opts["zero_bias"]:
                        if opts["kt_act"]:
                            nc.scalar.copy(out=kt[:, cs], in_=pk[HD : 2 * HD, :])
                        else:
                            nc.vector.tensor_copy(
                                out=kt[:, cs], in_=pk[HD : 2 * HD, :]
                            )
                    else:
                        if opts["kt_act"]:
                            nc.scalar.add(
                                out=kt[:, cs],
                                in_=pk[HD : 2 * HD, :],
                                add=bqk_sb[HD : 2 * HD],
                            )
                        else:
                            nc.vector.tensor_scalar_add(
                                out=kt[:, cs],
                                in0=pk[HD : 2 * HD, :],
                                scalar1=bqk_sb[HD : 2 * HD],
                            )
                    nc.tensor.matmul(
                        out=pk[0:HD, :], lhsT=wq_sb[:, :],
                        rhs=xp_t[:, cs], start=True, stop=True,
                    )
                    if opts["zero_bias"]:
                        nc.vector.tensor_copy(out=qt[:, cs], in_=pk[0:HD, :])
                    else:
                        nc.vector.tensor_scalar_add(
                            out=qt[:, cs],
                            in0=pk[0:HD, :],
                            scalar1=bqk_sb[0:HD],
                        )
                # gate tile arrives pre-broadcast from the host
                wb = singles.tile([128, L], FP16, tag=f"WB{b}")
                for half in range(2):
                    c0 = half * 1024
                    nc.sync.dma_start(
                        out=wb[:, c0 : c0 + 1024], in_=wbrev[b, :, c0 : c0 + 1024]
                    )
                return qt, kt, wb

            def emit_post_row(b, j, st, u, res):
                """Pool one 512-col segment (row at partition 32j).

                st[32j, t] = S'[512j + t - 1] (haloed; t in [0, 514)), so
                    u[t]   = st[t] + st[t+1]
                    res[c] = u[c] + st[c+2]  -> pooled[512j + c]
                All ops stay on partition 32j (engine ops need matching
                quadrant start partitions).
                """
                p = slice(32 * j, 32 * j + 1)
                nc.vector.tensor_add(
                    out=u[p, 0:513], in0=st[p, 0:513], in1=st[p, 1:514]
                )
                nc.vector.tensor_add(
                    out=res[p, 0:512], in0=u[p, 0:512], in1=st[p, 2:514]
                )

            def emit_instance():
                bqk_sb = singles.tile([2 * HD, 1], FP32, tag="bqk_sb")
                if not opts["zero_bias"]:
                    nc.sync.dma_start(out=bqk_sb, in_=bqk[:, :])
                wqk_sb = singles.tile([D, 2 * HD], FP16, tag="wqk_sb")
                nc.sync.dma_start(out=wqk_sb, in_=wqk[:, :])
                weights = (bqk_sb, wqk_sb)

                QT, KT, WB = {}, {}, {}
                REDUCE_DELAY = opts["reduce_delay"]

                QT[0], KT[0], WB[0] = emit_prep(0, weights)
                QT[1], KT[1], WB[1] = emit_prep(1, weights)

                for b in range(B):
                    # haloed psum accumulator: row 32j col t holds
                    # S'[512j + t - 1] for t in [0, 514); the never-written
                    # edge cols (row0 t=0, row3 t=513) are zeroed up front
                    s4 = s_pool.tile([97, 516], FP32, tag="S4")
                    nc.vector.memset(s4[0:1, 0:1], 0.0)
                    nc.vector.memset(s4[96:97, 513:514], 0.0)
                    st = post_pool.tile([97, 514], FP32, tag="St")
                    u4 = post_pool.tile([97, 513], FP32, tag="u4")
                    res = post_pool.tile([97, 512], FP32, tag="res")
                    # row j's psum group closes one block after segment j's
                    # data cols (the 1-col left-halo tail), except row 0
                    rmax = [15, 12, 8, 4]
                    pending = []

                    def emit_reduce(item, s4=s4, st=st, u4=u4, res=res,
                                    rmax=rmax, b=b):
                        rr, rt_t, dr16, rlen_r = item
                        for j in range(4):
                            g0 = 512 * j - 1      # global col of local t=0
                            glo = max(0, g0)
                            ghi = min(rlen_r, 512 * j + 513)
                            if ghi <= glo:
                                continue
                            t0 = glo - g0
                            t1 = ghi - g0
                            # split at the psum bank boundary (col 512): each
                            # bank must hold exactly one accumulation group,
                            # stopping at the last block that writes it (this
                            # also keeps the moving free dim <= 512)
                            cuts = [t0]
                            if t0 < 512 < t1:
                                cuts.append(512)
                            cuts.append(t1)
                            for c0_, c1_ in zip(cuts[:-1], cuts[1:]):
                                nc.tensor.matmul(
                                    out=s4[32 * j : 32 * j + 1, c0_:c1_],
                                    lhsT=dr16[:, 0:1],
                                    rhs=rt_t[:, g0 + c0_ : g0 + c1_],
                                    start=(rr == 0),
                                    stop=(rr == (2047 - (g0 + c0_)) // 128),
                                    tile_position=(0, 32 * j),
                                )
                            if rr == rmax[j]:
                                # row j closed: move to SBUF (frees nothing
                                # yet, but hides the copy + pool off the
                                # tail for rows 1..3)
                                p = slice(32 * j, 32 * j + 1)
                                nc.vector.tensor_copy(
                                    out=st[p, :], in_=s4[p, 0:514]
                                )
                                emit_post_row(b, j, st, u4, res)

                    for r in range(NBLK):
                        i0 = r * 128
                        ew = ew_pool.tile([128, W], FP16, tag="ew")
                        if opts["memset_pool"]:
                            nc.gpsimd.memset(ew[:, L:W], 0.0)
                        else:
                            nc.vector.memset(ew[:, L:W], 0.0)
                        dcol = []
                        for half in range(2):
                            c0 = half * 1024
                            ps = ps_pool.tile([128, 1024], FP32, tag="ps")
                            for j in range(2):
                                nc.tensor.matmul(
                                    out=ps[:, j * 512 : (j + 1) * 512],
                                    lhsT=QT[b][:, i0 : i0 + 128],
                                    rhs=KT[b][:, c0 + j * 512 : c0 + (j + 1) * 512],
                                    start=True,
                                    stop=True,
                                )
                            dc = small.tile([128, 1], FP32, tag="dc")
                            nc.scalar.activation(
                                out=ew[:, c0 : c0 + 1024],
                                in_=ps[:, :],
                                func=mybir.ActivationFunctionType.Exp,
                                scale=SCALE,
                                accum_out=dc,
                            )
                            dcol.append(dc)
                        dsum = small.tile([128, 1], FP32, tag="ds")
                        nc.vector.tensor_add(out=dsum, in0=dcol[0], in1=dcol[1])
                        dr32 = small.tile([128, 1], FP32, tag="dr32")
                        nc.vector.reciprocal_approx_fast(out=dr32, in_=dsum)
                        drecip16 = small.tile([128, 1], FP16, tag="dr16")
                        with nc.allow_low_precision("1/(3d) in fp16; washes out"):
                            nc.vector.tensor_scalar_mul(
                                out=drecip16, in0=dr32, scalar1=1.0 / 3.0
                            )

                        # gate by w (columns i0..L of this block: upper triangle)
                        nc.vector.tensor_mul(
                            out=ew[:, i0:L],
                            in0=ew[:, i0:L],
                            in1=WB[b][:, i0:L],
                        )

                        # skewed write: scratch[i0+p, c - i0 - p] = ew[p, c]
                        # flat dst = i0*W + p*(W-1) + (c - i0)
                        wlen = (L + 127) - i0
                        dst = bass.AP(
                            tensor=scr[b],
                            offset=i0 * W,
                            ap=[[W - 1, 128], [1, wlen]],
                        )
                        weng = nc.gpsimd if opts["write_gpsimd"] else nc.sync
                        wr = weng.dma_start(out=dst, in_=ew[:, i0 : L + 127])
                        if (b, r) in prev_rd:
                            add_dep_helper(
                                wr.ins, prev_rd[(b, r)], True,
                                "scr WAR vs prev rep",
                            )

                        # plain read back: rows i0..i0+128, cols 0..L-i0
                        rlen = L - i0
                        src = bass.AP(
                            tensor=scr[b],
                            offset=i0 * W,
                            ap=[[W, 128], [1, rlen]],
                        )
                        rt = rt_pool.tile([128, L], FP16, tag="rt")
                        reng = nc.scalar if opts["read_act"] else nc.sync
                        rd = reng.dma_start(out=rt[:, 0:rlen], in_=src)
                        add_dep_helper(rd.ins, wr.ins, True, "scratch RAW")
                        prev_rd[(b, r)] = rd.ins

                        # S'[m] += sum_p (1/(3 d[i0+p])) * rt[p, m] -- emitted a
                        # few blocks late so PE isn't head-of-line blocked on
                        # the scratch round-trip
                        pending.append((r, rt, drecip16, rlen))
                        if len(pending) > REDUCE_DELAY:
                            emit_reduce(pending.pop(0))

                    while pending:
                        emit_reduce(pending.pop(0))

                    nc.sync.dma_start(out=out[b, :], in_=res[0:97:32, :])

            for _rep in range(repeat):
                emit_instance()

    nc.finalize()
    return nc


_RUNNERS = {}


def _get_runner(repeat=1, **opts_kw):
    key = (repeat, tuple(sorted(opts_kw.items())))
    if key in _RUNNERS:
        return _RUNNERS[key]
    import jax
    from jax.experimental.shard_map import shard_map
    from jax.sharding import Mesh, PartitionSpec

    from concourse import bass2jax

    nc = build_nc(repeat, **opts_kw)
    bass2jax.install_neuronx_cc_hook()

    partition_name = nc.partition_id_tensor.name if nc.partition_id_tensor else None
    in_names, out_names, out_avals = [], [], []
    for alloc in nc.m.functions[0].allocations:
        if not isinstance(alloc, mybir.MemoryLocationSet):
            continue
        name = alloc.memorylocations[0].name
        if alloc.kind == "ExternalInput":
            if name != partition_name:
                in_names.append(name)
        elif alloc.kind == "ExternalOutput":
            out_names.append(name)
            out_avals.append(
                jax.core.ShapedArray(
                    tuple(alloc.tensor_shape), mybir.dt.np(alloc.dtype)
                )
            )
    n_params = len(in_names)
    n_outs = len(out_names)
    all_in = list(in_names) + list(out_names)
    if partition_name is not None:
        all_in.append(partition_name)

    def _body(*args):
        operands = list(args)
        if partition_name is not None:
            operands.append(bass2jax.partition_id_tensor())
        outs = bass2jax._bass_exec_p.bind(
            *operands,
            out_avals=tuple(out_avals),
            in_names=tuple(all_in),
            out_names=tuple(out_names),
            lowering_input_output_aliases=(),
            sim_require_finite=True,
            sim_require_nnan=True,
            nc=nc,
        )
        return tuple(outs)

    devices = jax.devices()[:H]
    mesh = Mesh(np.asarray(devices), ("core",))
    sharded = jax.jit(
        shard_map(
            _body,
            mesh=mesh,
            in_specs=(PartitionSpec("core"),) * (n_params + n_outs),
            out_specs=(PartitionSpec("core"),) * n_outs,
            check_rep=False,
        ),
        donate_argnums=tuple(range(n_params, n_params + n_outs)),
        keep_unused=True,
    )
    runner = (sharded, in_names, out_names, out_avals)
    _RUNNERS[key] = runner
    return runner


def _prep_in_maps(x, pe, Wq, bq, Wk, bk, Wv):
    x = np.asarray(x, np.float32)
    pe = np.asarray(pe, np.float32)
    Wq = np.asarray(Wq, np.float32)
    bq = np.asarray(bq, np.float32)
    Wk = np.asarray(Wk, np.float32)
    bk = np.asarray(bk, np.float32)
    Wv = np.asarray(Wv, np.float32)

    xp = x + pe[None, :, :]
    xpT = np.ascontiguousarray(xp.transpose(0, 2, 1)).astype(np.float16)
    # gate w[k] = sigmoid(x @ Wv)[L-1-k, h], broadcast to 128 partitions
    vlog = np.einsum("bld,dh->blh", x.astype(np.float64), Wv.astype(np.float64))
    vsig = 1.0 / (1.0 + np.exp(-vlog))          # (B, L, H)
    vrev = vsig[:, ::-1, :]                      # (B, L, H) reversed over l

    in_maps = []
    for h in range(H):
        hs = slice(h * HD, (h + 1) * HD)
        bqk = np.concatenate([bq[hs], bk[hs]]).reshape(2 * HD, 1)
        wb = np.ascontiguousarray(
            np.broadcast_to(vrev[:, None, :, h], (B, 128, L))
        ).astype(np.float16)
        wqk = np.concatenate([Wq[:, hs], Wk[:, hs]], axis=1)
        in_maps.append(
            {
                "xpT": xpT,
                "wbrev": wb,
                "wqk": np.ascontiguousarray(wqk).astype(np.float16),
                "bqk": np.ascontiguousarray(bqk).astype(np.float32),
            }
        )
    return in_maps


def run(in_maps, repeat=1, **opts_kw):
    sharded, in_names, out_names, out_avals = _get_runner(repeat, **opts_kw)
    concat_in = [
        np.concatenate([np.asarray(in_maps[c][n]) for c in range(H)], axis=0)
        for n in in_names
    ]
    concat_zeros = [
        np.zeros((H * a.shape[0], *a.shape[1:]), a.dtype) for a in out_avals
    ]
    out_arrs = sharded(*concat_in, *concat_zeros)
    return [
        {
            n: np.asarray(out_arrs[i]).reshape(H, *out_avals[i].shape)[c]
            for i, n in enumerate(out_names)
        }
        for c in range(H)
    ]


def kernel(x, pe, Wq, bq, Wk, bk, Wv):
    in_maps = _prep_in_maps(x, pe, Wq, bq, Wk, bk, Wv)
    zb = not (np.any(np.asarray(bq)) or np.any(np.asarray(bk)))
    results = run(in_maps, repeat=1, zero_bias=bool(zb))
    res = np.stack([results[h]["out"] for h in range(H)], axis=2)
    # window-3 SAME avg-pool divisor: ends divide by 2, not 3 (the kernel
    # pre-folds 1/3 into the reduce weights, so the two edge columns need
    # a 3/2 correction)
    res[:, 0, :] *= 1.5
    res[:, L - 1, :] *= 1.5
    return res


# revision 47
# speedup vs baseline: 1.2293x; 1.0050x over previous
"""Trainium2 Bass kernel for nn_MultiHeadDistanceLayer (sparse_attention).

Math: for each (head h, batch b) the reference collapses to
    S[m] = sum_k attn[k-m, k] * w[k],   w[k] = sigmoid(x @ Wv)[L-1-k, h]
(weighted superdiagonal sums of the attention matrix), followed by a
window-3 same-padded average pool over m (padding excluded from the
divisor):  out[b, m, h] = (S[m-1] + S[m] + S[m+1]) / cnt[m].

Sharding: 8 heads -> 8 NeuronCores; each core computes its head for both
batches.  Per (h, b) the kernel runs flash-style over 16 row blocks of 128
queries: scores via PE (K=HD=32), exp on ACT (with free row-sum accum ->
softmax denominators d), gating by w on DVE, then a *skewed* DMA write of
the probability block to a DRAM scratch so that superdiagonal m lands at
column m of every row.  A plain strided read back + a [1/(3d)]-weighted
ones-matmul on PE reduces partitions, PSUM-accumulating S/3 into a single
PSUM bank laid out as 4 rows at partitions {0,32,64,96} (512 cols each).
The window-3 pool then runs on DVE as 4-partition-parallel strided ops
(the /3 divisor is pre-folded; the two boundary elements get *1.5).
The sigmoid gate w is precomputed on the host (O(L*D) prep, like x+pe)
and arrives pre-broadcast as a [128, L] tile per batch, which removes the
sigmoid/exp ACT-table swap and the reversed-x input entirely.
"""

import contextlib

import numpy as np

import concourse.bacc as bacc
import concourse.bass as bass
import concourse.tile as tile
from concourse import mybir
from concourse.tile import add_dep_helper

B, L, D, H, HD, WIN = 2, 2048, 128, 8, 32, 3
NBLK = L // 128           # 16 row blocks per (h, b)
W = L + 128               # scratch row width (elements)
SCALE = float(HD) ** -0.5

FP16 = mybir.dt.float16
FP32 = mybir.dt.float32

DEFAULT_OPTS = dict(
    ew_bufs=10,
    rt_bufs=10,
    zero_bias=False,     # biases known to be zero -> plain copy instead of add
    kt_act=False,        # prep K-copies on DVE (ACT stays exp-only)
    reduce_delay=6,
    memset_pool=False,   # ew tail memset on DVE
    small_bufs=8,
    read_act=False,      # scratch read-back DMAs issued from ACT queue
    kt_act_b0=False,     # b0 prep K-copies on ACT
    row0_piece=True,     # piecewise row-0 pool
    write_gpsimd=False,  # scratch writes via SWDGE (Pool) instead of HWDGE
)


def build_nc(repeat=1, **opts_kw):
    opts = dict(DEFAULT_OPTS, **opts_kw)
    nc = bacc.Bacc("TRN2", target_bir_lowering=False, debug=False)

    xpT = nc.dram_tensor("xpT", [B, D, L], FP16, kind="ExternalInput")
    wbrev = nc.dram_tensor("wbrev", [B, 128, L], FP16, kind="ExternalInput")
    wqk = nc.dram_tensor("wqk", [D, 2 * HD], FP16, kind="ExternalInput")
    bqk = nc.dram_tensor("bqk", [2 * HD, 1], FP32, kind="ExternalInput")
    out = nc.dram_tensor("out", [B, L], FP32, kind="ExternalOutput")
    # one flat scratch region per (h, b) pair; row i of the logical [L, W]
    # grid holds the skew-shifted probability row i
    scr = [
        nc.dram_tensor(f"scr{b}", [L * W], FP16, kind="Internal") for b in range(B)
    ]

    with tile.TileContext(nc) as tc:
        with contextlib.ExitStack() as ctx:
            singles = ctx.enter_context(tc.tile_pool(name="singles", bufs=1))
            small = ctx.enter_context(tc.tile_pool(name="small", bufs=opts["small_bufs"]))
            ew_pool = ctx.enter_context(tc.tile_pool(name="ew", bufs=opts["ew_bufs"]))
            rt_pool = ctx.enter_context(tc.tile_pool(name="rt", bufs=opts["rt_bufs"]))
            ps_pool = ctx.enter_context(tc.tile_pool(name="ps", bufs=2, space="PSUM"))
            prep_pool = ctx.enter_context(tc.tile_pool(name="prep", bufs=2, space="PSUM"))
            s_pool = ctx.enter_context(tc.tile_pool(name="spsum", bufs=1, space="PSUM"))
            post_pool = ctx.enter_context(tc.tile_pool(name="post", bufs=2))

            # exp table warm-up: no data deps, so it schedules immediately and
            # the single act-table load lands before any real exp
            warm = singles.tile([1, 8], FP32)
            nc.vector.memset(warm, 0.0)
            warm2 = singles.tile([1, 8], FP32)
            nc.scalar.activation(out=warm2, in_=warm,
                                 func=mybir.ActivationFunctionType.Exp)

            prev_rd = {}
            # row 0 of the S accumulator finalizes column-span [a, b) after
            # block rr: block rr+1 only rewrites cols [1, rlen+1)
            ROW0_PIECES = {12: (385, 514), 13: (257, 385),
                           14: (129, 257), 15: (0, 129)}

            def emit_prep(b, weights):
                """QT/KT [32, L] fp16 and gate tile WB [128, L] fp16 for b.

                `slot` maps a global 512-col psum chunk index into the shared
                6-slot score region.  K is computed/copied first per chunk:
                the first score matmul needs both KT chunks of half 0 but
                only the first QT chunk, so K is the critical path.
                """
                bqk_sb, wqk_sb, xp_t = weights
                wq_sb = wqk_sb[:, 0:HD]
                wk_sb = wqk_sb[:, HD : 2 * HD]

                qt = singles.tile([HD, L], FP16, tag=f"QT{b}")
                kt = singles.tile([HD, L], FP16, tag=f"KT{b}")
                # b0's kt copies go on ACT (idle until the first exp); b1's
                # stay on DVE so they don't delay the early exp stream
                kt_act = opts["kt_act"] or (b == 0 and opts["kt_act_b0"])
                # packed prep psum: Q at partitions [0:32), K at [32:64)
                for ch in range(4):
                    cs = slice(ch * 512, (ch + 1) * 512)
                    pk = prep_pool.tile([128, 512], FP32, tag="prep")
                    nc.tensor.matmul(
                        out=pk[HD : 2 * HD, :], lhsT=wk_sb[:, :],
                        rhs=xp_t[:, cs], start=True, stop=True,
                    )
                    if opts["zero_bias"]:
                        if kt_act:
                            nc.scalar.copy(out=kt[:, cs], in_=pk[HD : 2 * HD, :])
                        else:
                            nc.vector.tensor_copy(
                                out=kt[:, cs], in_=pk[HD : 2 * HD, :]
                            )
                    else:
                        if kt_act:
                            nc.scalar.add(
                                out=kt[:, cs],
                                in_=pk[HD : 2 * HD, :],
                                add=bqk_sb[HD : 2 * HD],
                            )
                        else:
                            nc.vector.tensor_scalar_add(
                                out=kt[:, cs],
                                in0=pk[HD : 2 * HD, :],
                                scalar1=bqk_sb[HD : 2 * HD],
                            )
                    nc.tensor.matmul(
                        out=pk[0:HD, :], lhsT=wq_sb[:, :],
                        rhs=xp_t[:, cs], start=True, stop=True,
                    )
                    if opts["zero_bias"]:
                        nc.vector.tensor_copy(out=qt[:, cs], in_=pk[0:HD, :])
                    else:
                        nc.vector.tensor_scalar_add(
                            out=qt[:, cs],
                            in0=pk[0:HD, :],
                            scalar1=bqk_sb[0:HD],
                        )
                # gate tile arrives pre-broadcast from the host
                wb = singles.tile([128, L], FP16, tag=f"WB{b}")
                for half in range(2):
                    c0 = half * 1024
                    nc.sync.dma_start(
                        out=wb[:, c0 : c0 + 1024], in_=wbrev[b, :, c0 : c0 + 1024]
                    )
                return qt, kt, wb

            def emit_post_row(b, j, st, u, res):
                """Pool one 512-col segment (row at partition 32j).

                st[32j, t] = S'[512j + t - 1] (haloed; t in [0, 514)), so
                    u[t]   = st[t] + st[t+1]
                    res[c] = u[c] + st[c+2]  -> pooled[512j + c]
                All ops stay on partition 32j (engine ops need matching
                quadrant start partitions).
                """
                p = slice(32 * j, 32 * j + 1)
                nc.vector.tensor_add(
                    out=u[p, 0:513], in0=st[p, 0:513], in1=st[p, 1:514]
                )
                nc.vector.tensor_add(
                    out=res[p, 0:512], in0=u[p, 0:512], in1=st[p, 2:514]
                )

            def emit_instance():
                # b0's xp load is the critical path to the first exp: issue
                # it before the (tiny) weight loads so it leads the DMA queue
                xps = {}
                for b in range(B):
                    xps[b] = singles.tile([D, L], FP16, tag=f"xpT{b}", name="xp_t")
                nc.sync.dma_start(out=xps[0][:, 0:1024], in_=xpT[0, :, 0:1024])
                wqk_sb = singles.tile([D, 2 * HD], FP16, tag="wqk_sb")
                nc.sync.dma_start(out=wqk_sb, in_=wqk[:, :])
                bqk_sb = singles.tile([2 * HD, 1], FP32, tag="bqk_sb")
                if not opts["zero_bias"]:
                    nc.sync.dma_start(out=bqk_sb, in_=bqk[:, :])
                nc.sync.dma_start(out=xps[0][:, 1024:L], in_=xpT[0, :, 1024:L])
                for c0, c1 in ((0, 1024), (1024, L)):
                    nc.sync.dma_start(out=xps[1][:, c0:c1], in_=xpT[1, :, c0:c1])

                QT, KT, WB = {}, {}, {}
                REDUCE_DELAY = opts["reduce_delay"]

                QT[0], KT[0], WB[0] = emit_prep(0, (bqk_sb, wqk_sb, xps[0]))
                QT[1], KT[1], WB[1] = emit_prep(1, (bqk_sb, wqk_sb, xps[1]))

                for b in range(B):
                    # haloed psum accumulator: row 32j col t holds
                    # S'[512j + t - 1] for t in [0, 514); the never-written
                    # edge cols (row0 t=0, row3 t=513) are zeroed up front
                    s4 = s_pool.tile([97, 516], FP32, tag="S4")
                    nc.vector.memset(s4[0:1, 0:1], 0.0)
                    nc.vector.memset(s4[96:97, 513:514], 0.0)
                    st = post_pool.tile([97, 514], FP32, tag="St")
                    u4 = post_pool.tile([97, 513], FP32, tag="u4")
                    res = post_pool.tile([97, 512], FP32, tag="res")
                    # row j's psum group closes one block after segment j's
                    # data cols (the 1-col left-halo tail), except row 0
                    rmax = [15, 12, 8, 4]
                    pending = []

                    def emit_reduce(item, s4=s4, st=st, u4=u4, res=res,
                                    rmax=rmax, b=b):
                        rr, rt_t, dr16, rlen_r = item
                        for j in range(4):
                            g0 = 512 * j - 1      # global col of local t=0
                            glo = max(0, g0)
                            ghi = min(rlen_r, 512 * j + 513)
                            if ghi <= glo:
                                continue
                            t0 = glo - g0
                            t1 = ghi - g0
                            # split at the psum bank boundary (col 512): each
                            # bank must hold exactly one accumulation group,
                            # stopping at the last block that writes it (this
                            # also keeps the moving free dim <= 512)
                            cuts = [t0]
                            if t0 < 512 < t1:
                                cuts.append(512)
                            cuts.append(t1)
                            for c0_, c1_ in zip(cuts[:-1], cuts[1:]):
                                nc.tensor.matmul(
                                    out=s4[32 * j : 32 * j + 1, c0_:c1_],
                                    lhsT=dr16[:, 0:1],
                                    rhs=rt_t[:, g0 + c0_ : g0 + c1_],
                                    start=(rr == 0),
                                    stop=(rr == (2047 - (g0 + c0_)) // 128),
                                    tile_position=(0, 32 * j),
                                )
                            if j == 0:
                                # row 0 finalizes col-span [a, bnd) after
                                # block rr (later blocks only rewrite lower
                                # cols), so pool it piecewise: the tail only
                                # keeps [0, 129)
                                pieces = (ROW0_PIECES if opts["row0_piece"] else {15: (0, 514)})
                                if rr in pieces:
                                    a, bnd = pieces[rr]
                                    p = slice(0, 1)
                                    nc.vector.tensor_copy(
                                        out=st[p, a:bnd], in_=s4[p, a:bnd]
                                    )
                                    bu = min(bnd, 513)
                                    nc.vector.tensor_add(
                                        out=u4[p, a:bu],
                                        in0=st[p, a:bu],
                                        in1=st[p, a + 1 : bu + 1],
                                    )
                                    br = min(bnd, 512)
                                    nc.vector.tensor_add(
                                        out=res[p, a:br],
                                        in0=u4[p, a:br],
                                        in1=st[p, a + 2 : br + 2],
                                    )
                            elif rr == rmax[j]:
                                # row j closed: copy to SBUF and pool (hidden
                                # off the tail for rows 1..3)
                                p = slice(32 * j, 32 * j + 1)
                                nc.vector.tensor_copy(
                                    out=st[p, :], in_=s4[p, 0:514]
                                )
                                emit_post_row(b, j, st, u4, res)

                    for r in range(NBLK):
                        i0 = r * 128
                        ew = ew_pool.tile([128, W], FP16, tag="ew")
                        if opts["memset_pool"]:
                            nc.gpsimd.memset(ew[:, L:W], 0.0)
                        else:
                            nc.vector.memset(ew[:, L:W], 0.0)
                        dcol = []
                        for half in range(2):
                            c0 = half * 1024
                            ps = ps_pool.tile([128, 1024], FP32, tag="ps")
                            for j in range(2):
                                nc.tensor.matmul(
                                    out=ps[:, j * 512 : (j + 1) * 512],
                                    lhsT=QT[b][:, i0 : i0 + 128],
                                    rhs=KT[b][:, c0 + j * 512 : c0 + (j + 1) * 512],
                                    start=True,
                                    stop=True,
                                )
                            dc = small.tile([128, 1], FP32, tag="dc")
                            nc.scalar.activation(
                                out=ew[:, c0 : c0 + 1024],
                                in_=ps[:, :],
                                func=mybir.ActivationFunctionType.Exp,
                                scale=SCALE,
                                accum_out=dc,
                            )
                            dcol.append(dc)
                        dsum = small.tile([128, 1], FP32, tag="ds")
                        nc.vector.tensor_add(out=dsum, in0=dcol[0], in1=dcol[1])
                        dr32 = small.tile([128, 1], FP32, tag="dr32")
                        nc.vector.reciprocal_approx_fast(out=dr32, in_=dsum)
                        drecip16 = small.tile([128, 1], FP16, tag="dr16")
                        with nc.allow_low_precision("1/(3d) in fp16; washes out"):
                            nc.vector.tensor_scalar_mul(
                                out=drecip16, in0=dr32, scalar1=1.0 / 3.0
                            )

                        # gate by w (columns i0..L of this block: upper triangle)
                        nc.vector.tensor_mul(
                            out=ew[:, i0:L],
                            in0=ew[:, i0:L],
                            in1=WB[b][:, i0:L],
                        )

                        # skewed write: scratch[i0+p, c - i0 - p] = ew[p, c]
                        # flat dst = i0*W + p*(W-1) + (c - i0)
                        wlen = (L + 127) - i0
                        dst = bass.AP(
                            tensor=scr[b],
                            offset=i0 * W,
                            ap=[[W - 1, 128], [1, wlen]],
                        )
                        weng = nc.gpsimd if opts["write_gpsimd"] else nc.sync
                        wr = weng.dma_start(out=dst, in_=ew[:, i0 : L + 127])
                        if (b, r) in prev_rd:
                            add_dep_helper(
                                wr.ins, prev_rd[(b, r)], True,
                                "scr WAR vs prev rep",
                            )

                        # plain read back: rows i0..i0+128, cols 0..L-i0
                        rlen = L - i0
                        src = bass.AP(
                            tensor=scr[b],
                            offset=i0 * W,
                            ap=[[W, 128], [1, rlen]],
                        )
                        rt = rt_pool.tile([128, L], FP16, tag="rt")
                        reng = nc.scalar if opts["read_act"] else nc.sync
                        rd = reng.dma_start(out=rt[:, 0:rlen], in_=src)
                        add_dep_helper(rd.ins, wr.ins, True, "scratch RAW")
                        prev_rd[(b, r)] = rd.ins

                        # S'[m] += sum_p (1/(3 d[i0+p])) * rt[p, m] -- emitted a
                        # few blocks late so PE isn't head-of-line blocked on
                        # the scratch round-trip
                        pending.append((r, rt, drecip16, rlen))
                        if len(pending) > REDUCE_DELAY:
                            emit_reduce(pending.pop(0))

                    while pending:
                        emit_reduce(pending.pop(0))

                    nc.sync.dma_start(out=out[b, :], in_=res[0:97:32, :])

            for _rep in range(repeat):
                emit_instance()

    nc.finalize()
    return nc


_RUNNERS = {}


def _get_runner(repeat=1, **opts_kw):
    key = (repeat, tuple(sorted(opts_kw.items())))
    if key in _RUNNERS:
        return _RUNNERS[key]
    import jax
    from jax.experimental.shard_map import shard_map
    from jax.sharding import Mesh, PartitionSpec

    from concourse import bass2jax

    nc = build_nc(repeat, **opts_kw)
    bass2jax.install_neuronx_cc_hook()

    partition_name = nc.partition_id_tensor.name if nc.partition_id_tensor else None
    in_names, out_names, out_avals = [], [], []
    for alloc in nc.m.functions[0].allocations:
        if not isinstance(alloc, mybir.MemoryLocationSet):
            continue
        name = alloc.memorylocations[0].name
        if alloc.kind == "ExternalInput":
            if name != partition_name:
                in_names.append(name)
        elif alloc.kind == "ExternalOutput":
            out_names.append(name)
            out_avals.append(
                jax.core.ShapedArray(
                    tuple(alloc.tensor_shape), mybir.dt.np(alloc.dtype)
                )
            )
    n_params = len(in_names)
    n_outs = len(out_names)
    all_in = list(in_names) + list(out_names)
    if partition_name is not None:
        all_in.append(partition_name)

    def _body(*args):
        operands = list(args)
        if partition_name is not None:
            operands.append(bass2jax.partition_id_tensor())
        outs = bass2jax._bass_exec_p.bind(
            *operands,
            out_avals=tuple(out_avals),
            in_names=tuple(all_in),
            out_names=tuple(out_names),
            lowering_input_output_aliases=(),
            sim_require_finite=True,
            sim_require_nnan=True,
            nc=nc,
        )
        return tuple(outs)

    devices = jax.devices()[:H]
    mesh = Mesh(np.asarray(devices), ("core",))
    sharded = jax.jit(
        shard_map(
            _body,
            mesh=mesh,
            in_specs=(PartitionSpec("core"),) * (n_params + n_outs),
            out_specs=(PartitionSpec("core"),) * n_outs,
            check_rep=False,
        ),
        donate_argnums=tuple(range(n_params, n_params + n_outs)),
        keep_unused=True,
    )
    runner = (sharded, in_names, out_names, out_avals)
    _RUNNERS[key] = runner
    return runner


def _prep_in_maps(x, pe, Wq, bq, Wk, bk, Wv):
    x = np.asarray(x, np.float32)
    pe = np.asarray(pe, np.float32)
    Wq = np.asarray(Wq, np.float32)
    bq = np.asarray(bq, np.float32)
    Wk = np.asarray(Wk, np.float32)
    bk = np.asarray(bk, np.float32)
    Wv = np.asarray(Wv, np.float32)

    xp = x + pe[None, :, :]
    xpT = np.ascontiguousarray(xp.transpose(0, 2, 1)).astype(np.float16)
    # gate w[k] = sigmoid(x @ Wv)[L-1-k, h], broadcast to 128 partitions
    vlog = np.einsum("bld,dh->blh", x.astype(np.float64), Wv.astype(np.float64))
    vsig = 1.0 / (1.0 + np.exp(-vlog))          # (B, L, H)
    vrev = vsig[:, ::-1, :]                      # (B, L, H) reversed over l

    in_maps = []
    for h in range(H):
        hs = slice(h * HD, (h + 1) * HD)
        bqk = np.concatenate([bq[hs], bk[hs]]).reshape(2 * HD, 1)
        wb = np.ascontiguousarray(
            np.broadcast_to(vrev[:, None, :, h], (B, 128, L))
        ).astype(np.float16)
        wqk = np.concatenate([Wq[:, hs], Wk[:, hs]], axis=1)
        in_maps.append(
            {
                "xpT": xpT,
                "wbrev": wb,
                "wqk": np.ascontiguousarray(wqk).astype(np.float16),
                "bqk": np.ascontiguousarray(bqk).astype(np.float32),
            }
        )
    return in_maps


def run(in_maps, repeat=1, **opts_kw):
    sharded, in_names, out_names, out_avals = _get_runner(repeat, **opts_kw)
    concat_in = [
        np.concatenate([np.asarray(in_maps[c][n]) for c in range(H)], axis=0)
        for n in in_names
    ]
    concat_zeros = [
        np.zeros((H * a.shape[0], *a.shape[1:]), a.dtype) for a in out_avals
    ]
    out_arrs = sharded(*concat_in, *concat_zeros)
    return [
        {
            n: np.asarray(out_arrs[i]).reshape(H, *out_avals[i].shape)[c]
            for i, n in enumerate(out_names)
        }
        for c in range(H)
    ]


def kernel(x, pe, Wq, bq, Wk, bk, Wv):
    in_maps = _prep_in_maps(x, pe, Wq, bq, Wk, bk, Wv)
    zb = not (np.any(np.asarray(bq)) or np.any(np.asarray(bk)))
    results = run(in_maps, repeat=1, zero_bias=bool(zb))
    res = np.stack([results[h]["out"] for h in range(H)], axis=2)
    # window-3 SAME avg-pool divisor: ends divide by 2, not 3 (the kernel
    # pre-folds 1/3 into the reduce weights, so the two edge columns need
    # a 3/2 correction)
    res[:, 0, :] *= 1.5
    res[:, L - 1, :] *= 1.5
    return res


# revision 50
# speedup vs baseline: 1.2430x; 1.0111x over previous
"""Trainium2 Bass kernel for nn_MultiHeadDistanceLayer (sparse_attention).

Math: for each (head h, batch b) the reference collapses to
    S[m] = sum_k attn[k-m, k] * w[k],   w[k] = sigmoid(x @ Wv)[L-1-k, h]
(weighted superdiagonal sums of the attention matrix), followed by a
window-3 same-padded average pool over m (padding excluded from the
divisor):  out[b, m, h] = (S[m-1] + S[m] + S[m+1]) / cnt[m].

Sharding: 8 heads -> 8 NeuronCores; each core computes its head for both
batches.  Per (h, b) the kernel runs flash-style over 16 row blocks of 128
queries: scores via PE (K=HD=32), exp on ACT (with free row-sum accum ->
softmax denominators d), gating by w on DVE, then a *skewed* DMA write of
the probability block to a DRAM scratch so that superdiagonal m lands at
column m of every row.  A plain strided read back + a [1/(3d)]-weighted
ones-matmul on PE reduces partitions, PSUM-accumulating S/3 into a single
PSUM bank laid out as 4 rows at partitions {0,32,64,96} (512 cols each).
The window-3 pool then runs on DVE as 4-partition-parallel strided ops
(the /3 divisor is pre-folded; the two boundary elements get *1.5).
The sigmoid gate w is precomputed on the host (O(L*D) prep, like x+pe)
and arrives pre-broadcast as a [128, L] tile per batch, which removes the
sigmoid/exp ACT-table swap and the reversed-x input entirely.
"""

import contextlib

import numpy as np

import concourse.bacc as bacc
import concourse.bass as bass
import concourse.tile as tile
from concourse import mybir
from concourse.tile import add_dep_helper

B, L, D, H, HD, WIN = 2, 2048, 128, 8, 32, 3
NBLK = L // 128           # 16 row blocks per (h, b)
W = L + 128               # scratch row width (elements)
SCALE = float(HD) ** -0.5

FP16 = mybir.dt.float16
FP32 = mybir.dt.float32

DEFAULT_OPTS = dict(
    ew_bufs=10,
    rt_bufs=10,
    zero_bias=False,     # biases known to be zero -> plain copy instead of add
    kt_act=False,        # prep K-copies on DVE (ACT stays exp-only)
    reduce_delay=6,
    memset_pool=False,   # ew tail memset on DVE
    small_bufs=8,
    read_act=False,      # scratch read-back DMAs issued from ACT queue
    kt_act_b0=False,     # b0 prep K-copies on ACT
    row0_piece=True,     # piecewise row-0 pool
    write_gpsimd=False,  # scratch writes via SWDGE (Pool) instead of HWDGE
)


def build_nc(repeat=1, **opts_kw):
    opts = dict(DEFAULT_OPTS, **opts_kw)
    nc = bacc.Bacc("TRN2", target_bir_lowering=False, debug=False)

    xpT = nc.dram_tensor("xpT", [B, D, L], FP16, kind="ExternalInput")
    wbrev = nc.dram_tensor("wbrev", [B, 128, L], FP16, kind="ExternalInput")
    wqk = nc.dram_tensor("wqk", [D, 2 * HD], FP16, kind="ExternalInput")
    bqk = nc.dram_tensor("bqk", [2 * HD, 1], FP32, kind="ExternalInput")
    out = nc.dram_tensor("out", [B, L], FP32, kind="ExternalOutput")
    # one flat scratch region per (h, b) pair; row i of the logical [L, W]
    # grid holds the skew-shifted probability row i
    scr = [
        nc.dram_tensor(f"scr{b}", [L * W], FP16, kind="Internal") for b in range(B)
    ]

    with tile.TileContext(nc) as tc:
        with contextlib.ExitStack() as ctx:
            singles = ctx.enter_context(tc.tile_pool(name="singles", bufs=1))
            small = ctx.enter_context(tc.tile_pool(name="small", bufs=opts["small_bufs"]))
            ew_pool = ctx.enter_context(tc.tile_pool(name="ew", bufs=opts["ew_bufs"]))
            rt_pool = ctx.enter_context(tc.tile_pool(name="rt", bufs=opts["rt_bufs"]))
            ps_pool = ctx.enter_context(tc.tile_pool(name="ps", bufs=2, space="PSUM"))
            prep_pool = ctx.enter_context(tc.tile_pool(name="prep", bufs=2, space="PSUM"))
            s_pool = ctx.enter_context(tc.tile_pool(name="spsum", bufs=1, space="PSUM"))
            post_pool = ctx.enter_context(tc.tile_pool(name="post", bufs=2))

            # exp table warm-up: no data deps, so it schedules immediately and
            # the single act-table load lands before any real exp
            warm = singles.tile([1, 8], FP32)
            nc.vector.memset(warm, 0.0)
            warm2 = singles.tile([1, 8], FP32)
            nc.scalar.activation(out=warm2, in_=warm,
                                 func=mybir.ActivationFunctionType.Exp)
            # PE p-state warm-up: ~3us of dummy matmuls so the prep/score
            # matmuls on the first-exp critical path run at full clock
            wml = singles.tile([128, 1], FP16)
            nc.vector.memset(wml, 0.0)
            wmr = singles.tile([128, 512], FP16)
            nc.vector.memset(wmr, 0.0)
            wps = prep_pool.tile([128, 512], FP32, tag="prep", name="wps")
            for _ in range(7):
                nc.tensor.matmul(
                    out=wps[0:1, :], lhsT=wml[:, 0:1], rhs=wmr[:, :],
                    start=True, stop=True,
                )

            prev_rd = {}
            # row 0 of the S accumulator finalizes column-span [a, b) after
            # block rr: block rr+1 only rewrites cols [1, rlen+1)
            ROW0_PIECES = {12: (385, 514), 13: (257, 385),
                           14: (129, 257), 15: (0, 129)}

            def emit_prep(b, weights):
                """QT/KT [32, L] fp16 and gate tile WB [128, L] fp16 for b.

                `slot` maps a global 512-col psum chunk index into the shared
                6-slot score region.  K is computed/copied first per chunk:
                the first score matmul needs both KT chunks of half 0 but
                only the first QT chunk, so K is the critical path.
                """
                bqk_sb, wqk_sb, xp_t = weights
                wq_sb = wqk_sb[:, 0:HD]
                wk_sb = wqk_sb[:, HD : 2 * HD]

                qt = singles.tile([HD, L], FP16, tag=f"QT{b}")
                kt = singles.tile([HD, L], FP16, tag=f"KT{b}")
                # b0's kt copies go on ACT (idle until the first exp); b1's
                # stay on DVE so they don't delay the early exp stream
                kt_act = opts["kt_act"] or (b == 0 and opts["kt_act_b0"])
                # packed prep psum: Q at partitions [0:32), K at [32:64)
                for ch in range(4):
                    cs = slice(ch * 512, (ch + 1) * 512)
                    pk = prep_pool.tile([128, 512], FP32, tag="prep")
                    nc.tensor.matmul(
                        out=pk[HD : 2 * HD, :], lhsT=wk_sb[:, :],
                        rhs=xp_t[:, cs], start=True, stop=True,
                    )
                    if opts["zero_bias"]:
                        if kt_act:
                            nc.scalar.copy(out=kt[:, cs], in_=pk[HD : 2 * HD, :])
                        else:
                            nc.vector.tensor_copy(
                                out=kt[:, cs], in_=pk[HD : 2 * HD, :]
                            )
                    else:
                        if kt_act:
                            nc.scalar.add(
                                out=kt[:, cs],
                                in_=pk[HD : 2 * HD, :],
                                add=bqk_sb[HD : 2 * HD],
                            )
                        else:
                            nc.vector.tensor_scalar_add(
                                out=kt[:, cs],
                                in0=pk[HD : 2 * HD, :],
                                scalar1=bqk_sb[HD : 2 * HD],
                            )
                    nc.tensor.matmul(
                        out=pk[0:HD, :], lhsT=wq_sb[:, :],
                        rhs=xp_t[:, cs], start=True, stop=True,
                    )
                    if opts["zero_bias"]:
                        nc.vector.tensor_copy(out=qt[:, cs], in_=pk[0:HD, :])
                    else:
                        nc.vector.tensor_scalar_add(
                            out=qt[:, cs],
                            in0=pk[0:HD, :],
                            scalar1=bqk_sb[0:HD],
                        )
                # gate tile arrives pre-broadcast from the host
                wb = singles.tile([128, L], FP16, tag=f"WB{b}")
                for half in range(2):
                    c0 = half * 1024
                    nc.sync.dma_start(
                        out=wb[:, c0 : c0 + 1024], in_=wbrev[b, :, c0 : c0 + 1024]
                    )
                return qt, kt, wb

            def emit_post_row(b, j, st, u, res):
                """Pool one 512-col segment (row at partition 32j).

                st[32j, t] = S'[512j + t - 1] (haloed; t in [0, 514)), so
                    u[t]   = st[t] + st[t+1]
                    res[c] = u[c] + st[c+2]  -> pooled[512j + c]
                All ops stay on partition 32j (engine ops need matching
                quadrant start partitions).
                """
                p = slice(32 * j, 32 * j + 1)
                nc.gpsimd.tensor_add(
                    out=u[p, 0:513], in0=st[p, 0:513], in1=st[p, 1:514]
                )
                nc.gpsimd.tensor_add(
                    out=res[p, 0:512], in0=u[p, 0:512], in1=st[p, 2:514]
                )

            def emit_instance():
                # b0's xp load is the critical path to the first exp: issue
                # it before the (tiny) weight loads so it leads the DMA queue
                xps = {}
                for b in range(B):
                    xps[b] = singles.tile([D, L], FP16, tag=f"xpT{b}", name="xp_t")
                nc.sync.dma_start(out=xps[0][:, 0:1024], in_=xpT[0, :, 0:1024])
                wqk_sb = singles.tile([D, 2 * HD], FP16, tag="wqk_sb")
                nc.sync.dma_start(out=wqk_sb, in_=wqk[:, :])
                bqk_sb = singles.tile([2 * HD, 1], FP32, tag="bqk_sb")
                if not opts["zero_bias"]:
                    nc.sync.dma_start(out=bqk_sb, in_=bqk[:, :])
                nc.sync.dma_start(out=xps[0][:, 1024:L], in_=xpT[0, :, 1024:L])
                for c0, c1 in ((0, 1024), (1024, L)):
                    nc.sync.dma_start(out=xps[1][:, c0:c1], in_=xpT[1, :, c0:c1])

                QT, KT, WB = {}, {}, {}
                REDUCE_DELAY = opts["reduce_delay"]

                QT[0], KT[0], WB[0] = emit_prep(0, (bqk_sb, wqk_sb, xps[0]))
                QT[1], KT[1], WB[1] = emit_prep(1, (bqk_sb, wqk_sb, xps[1]))

                for b in range(B):
                    # haloed psum accumulator: row 32j col t holds
                    # S'[512j + t - 1] for t in [0, 514); the never-written
                    # edge cols (row0 t=0, row3 t=513) are zeroed up front
                    s4 = s_pool.tile([97, 516], FP32, tag="S4")
                    nc.vector.memset(s4[0:1, 0:1], 0.0)
                    nc.vector.memset(s4[96:97, 513:514], 0.0)
                    st = post_pool.tile([97, 514], FP32, tag="St")
                    u4 = post_pool.tile([97, 513], FP32, tag="u4")
                    res = post_pool.tile([97, 512], FP32, tag="res")
                    # row j's psum group closes one block after segment j's
                    # data cols (the 1-col left-halo tail), except row 0
                    rmax = [15, 12, 8, 4]
                    pending = []

                    def emit_reduce(item, s4=s4, st=st, u4=u4, res=res,
                                    rmax=rmax, b=b):
                        rr, rt_t, dr16, rlen_r = item
                        for j in range(4):
                            g0 = 512 * j - 1      # global col of local t=0
                            glo = max(0, g0)
                            ghi = min(rlen_r, 512 * j + 513)
                            if ghi <= glo:
                                continue
                            t0 = glo - g0
                            t1 = ghi - g0
                            # split at the psum bank boundary (col 512): each
                            # bank must hold exactly one accumulation group,
                            # stopping at the last block that writes it (this
                            # also keeps the moving free dim <= 512)
                            cuts = [t0]
                            if t0 < 512 < t1:
                                cuts.append(512)
                            cuts.append(t1)
                            for c0_, c1_ in zip(cuts[:-1], cuts[1:]):
                                nc.tensor.matmul(
                                    out=s4[32 * j : 32 * j + 1, c0_:c1_],
                                    lhsT=dr16[:, 0:1],
                                    rhs=rt_t[:, g0 + c0_ : g0 + c1_],
                                    start=(rr == 0),
                                    stop=(rr == (2047 - (g0 + c0_)) // 128),
                                    tile_position=(0, 32 * j),
                                )
                            if j == 0:
                                # row 0 finalizes col-span [a, bnd) after
                                # block rr (later blocks only rewrite lower
                                # cols), so pool it piecewise: the tail only
                                # keeps [0, 129)
                                pieces = (ROW0_PIECES if opts["row0_piece"] else {15: (0, 514)})
                                if rr in pieces:
                                    a, bnd = pieces[rr]
                                    eng = nc.vector if rr == 15 else nc.gpsimd
                                    p = slice(0, 1)
                                    nc.vector.tensor_copy(
                                        out=st[p, a:bnd], in_=s4[p, a:bnd]
                                    )
                                    bu = min(bnd, 513)
                                    eng.tensor_add(
                                        out=u4[p, a:bu],
                                        in0=st[p, a:bu],
                                        in1=st[p, a + 1 : bu + 1],
                                    )
                                    br = min(bnd, 512)
                                    eng.tensor_add(
                                        out=res[p, a:br],
                                        in0=u4[p, a:br],
                                        in1=st[p, a + 2 : br + 2],
                                    )
                            elif rr == rmax[j]:
                                # row j closed: copy to SBUF and pool (hidden
                                # off the tail for rows 1..3)
                                p = slice(32 * j, 32 * j + 1)
                                nc.vector.tensor_copy(
                                    out=st[p, :], in_=s4[p, 0:514]
                                )
                                emit_post_row(b, j, st, u4, res)

                    for r in range(NBLK):
                        i0 = r * 128
                        ew = ew_pool.tile([128, W], FP16, tag="ew")
                        if opts["memset_pool"]:
                            nc.gpsimd.memset(ew[:, L:W], 0.0)
                        else:
                            nc.vector.memset(ew[:, L:W], 0.0)
                        dcol = []
                        for half in range(2):
                            c0 = half * 1024
                            ps = ps_pool.tile([128, 1024], FP32, tag="ps")
                            for j in range(2):
                                nc.tensor.matmul(
                                    out=ps[:, j * 512 : (j + 1) * 512],
                                    lhsT=QT[b][:, i0 : i0 + 128],
                                    rhs=KT[b][:, c0 + j * 512 : c0 + (j + 1) * 512],
                                    start=True,
                                    stop=True,
                                )
                            dc = small.tile([128, 1], FP32, tag="dc")
                            nc.scalar.activation(
                                out=ew[:, c0 : c0 + 1024],
                                in_=ps[:, :],
                                func=mybir.ActivationFunctionType.Exp,
                                scale=SCALE,
                                accum_out=dc,
                            )
                            dcol.append(dc)
                        dsum = small.tile([128, 1], FP32, tag="ds")
                        nc.vector.tensor_add(out=dsum, in0=dcol[0], in1=dcol[1])
                        dr32 = small.tile([128, 1], FP32, tag="dr32")
                        nc.vector.reciprocal_approx_fast(out=dr32, in_=dsum)
                        drecip16 = small.tile([128, 1], FP16, tag="dr16")
                        with nc.allow_low_precision("1/(3d) in fp16; washes out"):
                            nc.vector.tensor_scalar_mul(
                                out=drecip16, in0=dr32, scalar1=1.0 / 3.0
                            )

                        # gate by w (columns i0..L of this block: upper triangle)
                        nc.vector.tensor_mul(
                            out=ew[:, i0:L],
                            in0=ew[:, i0:L],
                            in1=WB[b][:, i0:L],
                        )

                        # skewed write: scratch[i0+p, c - i0 - p] = ew[p, c]
                        # flat dst = i0*W + p*(W-1) + (c - i0)
                        wlen = (L + 127) - i0
                        dst = bass.AP(
                            tensor=scr[b],
                            offset=i0 * W,
                            ap=[[W - 1, 128], [1, wlen]],
                        )
                        weng = nc.gpsimd if opts["write_gpsimd"] else nc.sync
                        wr = weng.dma_start(out=dst, in_=ew[:, i0 : L + 127])
                        if (b, r) in prev_rd:
                            add_dep_helper(
                                wr.ins, prev_rd[(b, r)], True,
                                "scr WAR vs prev rep",
                            )

                        # plain read back: rows i0..i0+128, cols 0..L-i0
                        rlen = L - i0
                        src = bass.AP(
                            tensor=scr[b],
                            offset=i0 * W,
                            ap=[[W, 128], [1, rlen]],
                        )
                        rt = rt_pool.tile([128, L], FP16, tag="rt")
                        reng = nc.scalar if opts["read_act"] else nc.sync
                        rd = reng.dma_start(out=rt[:, 0:rlen], in_=src)
                        add_dep_helper(rd.ins, wr.ins, True, "scratch RAW")
                        prev_rd[(b, r)] = rd.ins

                        # S'[m] += sum_p (1/(3 d[i0+p])) * rt[p, m] -- emitted a
                        # few blocks late so PE isn't head-of-line blocked on
                        # the scratch round-trip
                        pending.append((r, rt, drecip16, rlen))
                        if len(pending) > REDUCE_DELAY:
                            emit_reduce(pending.pop(0))

                    while pending:
                        emit_reduce(pending.pop(0))

                    nc.sync.dma_start(out=out[b, :], in_=res[0:97:32, :])

            for _rep in range(repeat):
                emit_instance()

    nc.finalize()
    return nc


_RUNNERS = {}


def _get_runner(repeat=1, **opts_kw):
    key = (repeat, tuple(sorted(opts_kw.items())))
    if key in _RUNNERS:
        return _RUNNERS[key]
    import jax
    from jax.experimental.shard_map import shard_map
    from jax.sharding import Mesh, PartitionSpec

    from concourse import bass2jax

    nc = build_nc(repeat, **opts_kw)
    bass2jax.install_neuronx_cc_hook()

    partition_name = nc.partition_id_tensor.name if nc.partition_id_tensor else None
    in_names, out_names, out_avals = [], [], []
    for alloc in nc.m.functions[0].allocations:
        if not isinstance(alloc, mybir.MemoryLocationSet):
            continue
        name = alloc.memorylocations[0].name
        if alloc.kind == "ExternalInput":
            if name != partition_name:
                in_names.append(name)
        elif alloc.kind == "ExternalOutput":
            out_names.append(name)
            out_avals.append(
                jax.core.ShapedArray(
                    tuple(alloc.tensor_shape), mybir.dt.np(alloc.dtype)
                )
            )
    n_params = len(in_names)
    n_outs = len(out_names)
    all_in = list(in_names) + list(out_names)
    if partition_name is not None:
        all_in.append(partition_name)

    def _body(*args):
        operands = list(args)
        if partition_name is not None:
            operands.append(bass2jax.partition_id_tensor())
        outs = bass2jax._bass_exec_p.bind(
            *operands,
            out_avals=tuple(out_avals),
            in_names=tuple(all_in),
            out_names=tuple(out_names),
            lowering_input_output_aliases=(),
            sim_require_finite=True,
            sim_require_nnan=True,
            nc=nc,
        )
        return tuple(outs)

    devices = jax.devices()[:H]
    mesh = Mesh(np.asarray(devices), ("core",))
    sharded = jax.jit(
        shard_map(
            _body,
            mesh=mesh,
            in_specs=(PartitionSpec("core"),) * (n_params + n_outs),
            out_specs=(PartitionSpec("core"),) * n_outs,
            check_rep=False,
        ),
        donate_argnums=tuple(range(n_params, n_params + n_outs)),
        keep_unused=True,
    )
    runner = (sharded, in_names, out_names, out_avals)
    _RUNNERS[key] = runner
    return runner


def _prep_in_maps(x, pe, Wq, bq, Wk, bk, Wv):
    x = np.asarray(x, np.float32)
    pe = np.asarray(pe, np.float32)
    Wq = np.asarray(Wq, np.float32)
    bq = np.asarray(bq, np.float32)
    Wk = np.asarray(Wk, np.float32)
    bk = np.asarray(bk, np.float32)
    Wv = np.asarray(Wv, np.float32)

    xp = x + pe[None, :, :]
    xpT = np.ascontiguousarray(xp.transpose(0, 2, 1)).astype(np.float16)
    # gate w[k] = sigmoid(x @ Wv)[L-1-k, h], broadcast to 128 partitions
    vlog = np.einsum("bld,dh->blh", x.astype(np.float64), Wv.astype(np.float64))
    vsig = 1.0 / (1.0 + np.exp(-vlog))          # (B, L, H)
    vrev = vsig[:, ::-1, :]                      # (B, L, H) reversed over l

    in_maps = []
    for h in range(H):
        hs = slice(h * HD, (h + 1) * HD)
        bqk = np.concatenate([bq[hs], bk[hs]]).reshape(2 * HD, 1)
        wb = np.ascontiguousarray(
            np.broadcast_to(vrev[:, None, :, h], (B, 128, L))
        ).astype(np.float16)
        wqk = np.concatenate([Wq[:, hs], Wk[:, hs]], axis=1)
        in_maps.append(
            {
                "xpT": xpT,
                "wbrev": wb,
                "wqk": np.ascontiguousarray(wqk).astype(np.float16),
                "bqk": np.ascontiguousarray(bqk).astype(np.float32),
            }
        )
    return in_maps


def run(in_maps, repeat=1, **opts_kw):
    sharded, in_names, out_names, out_avals = _get_runner(repeat, **opts_kw)
    concat_in = [
        np.concatenate([np.asarray(in_maps[c][n]) for c in range(H)], axis=0)
        for n in in_names
    ]
    concat_zeros = [
        np.zeros((H * a.shape[0], *a.shape[1:]), a.dtype) for a in out_avals
    ]
    out_arrs = sharded(*concat_in, *concat_zeros)
    return [
        {
            n: np.asarray(out_arrs[i]).reshape(H, *out_avals[i].shape)[c]
            for i, n in enumerate(out_names)
        }
        for c in range(H)
    ]


def kernel(x, pe, Wq, bq, Wk, bk, Wv):
    in_maps = _prep_in_maps(x, pe, Wq, bq, Wk, bk, Wv)
    zb = not (np.any(np.asarray(bq)) or np.any(np.asarray(bk)))
    results = run(in_maps, repeat=1, zero_bias=bool(zb))
    res = np.stack([results[h]["out"] for h in range(H)], axis=2)
    # window-3 SAME avg-pool divisor: ends divide by 2, not 3 (the kernel
    # pre-folds 1/3 into the reduce weights, so the two edge columns need
    # a 3/2 correction)
    res[:, 0, :] *= 1.5
    res[:, L - 1, :] *= 1.5
    return res


# revision 52
# speedup vs baseline: 1.2551x; 1.0097x over previous
"""Trainium2 Bass kernel for nn_MultiHeadDistanceLayer (sparse_attention).

Math: for each (head h, batch b) the reference collapses to
    S[m] = sum_k attn[k-m, k] * w[k],   w[k] = sigmoid(x @ Wv)[L-1-k, h]
(weighted superdiagonal sums of the attention matrix), followed by a
window-3 same-padded average pool over m (padding excluded from the
divisor):  out[b, m, h] = (S[m-1] + S[m] + S[m+1]) / cnt[m].

Sharding: 8 heads -> 8 NeuronCores; each core computes its head for both
batches.  Per (h, b) the kernel runs flash-style over 16 row blocks of 128
queries: scores via PE (K=HD=32), exp on ACT (with free row-sum accum ->
softmax denominators d), gating by w on DVE, then a *skewed* DMA write of
the probability block to a DRAM scratch so that superdiagonal m lands at
column m of every row.  A plain strided read back + a [1/(3d)]-weighted
ones-matmul on PE reduces partitions, PSUM-accumulating S/3 into a single
PSUM bank laid out as 4 rows at partitions {0,32,64,96} (512 cols each).
The window-3 pool then runs on DVE as 4-partition-parallel strided ops
(the /3 divisor is pre-folded; the two boundary elements get *1.5).
The sigmoid gate w is precomputed on the host (O(L*D) prep, like x+pe)
and arrives pre-broadcast as a [128, L] tile per batch, which removes the
sigmoid/exp ACT-table swap and the reversed-x input entirely.
"""

import contextlib

import numpy as np

import concourse.bacc as bacc
import concourse.bass as bass
import concourse.tile as tile
from concourse import mybir
from concourse.tile import add_dep_helper

B, L, D, H, HD, WIN = 2, 2048, 128, 8, 32, 3
NBLK = L // 128           # 16 row blocks per (h, b)
W = L + 128               # scratch row width (elements)
SCALE = float(HD) ** -0.5

FP16 = mybir.dt.float16
FP32 = mybir.dt.float32

DEFAULT_OPTS = dict(
    ew_bufs=10,
    rt_bufs=10,
    zero_bias=False,     # biases known to be zero -> plain copy instead of add
    kt_act=False,        # prep K-copies on DVE (ACT stays exp-only)
    reduce_delay=6,
    memset_pool=False,   # ew tail memset on DVE
    small_bufs=8,
    read_act=False,      # scratch read-back DMAs issued from ACT queue
    kt_act_b0=False,     # b0 prep K-copies on ACT
    row0_piece=True,     # piecewise row-0 pool
    write_gpsimd=False,  # scratch writes via SWDGE (Pool) instead of HWDGE
)


def build_nc(repeat=1, **opts_kw):
    opts = dict(DEFAULT_OPTS, **opts_kw)
    nc = bacc.Bacc("TRN2", target_bir_lowering=False, debug=False)

    xpT = nc.dram_tensor("xpT", [B, D, L], FP16, kind="ExternalInput")
    wbrev = nc.dram_tensor("wbrev", [B, 128, L], FP16, kind="ExternalInput")
    wqk = nc.dram_tensor("wqk", [D, 2 * HD], FP16, kind="ExternalInput")
    bqk = nc.dram_tensor("bqk", [2 * HD, 1], FP32, kind="ExternalInput")
    out = nc.dram_tensor("out", [B, L], FP32, kind="ExternalOutput")
    # one flat scratch region per (h, b) pair; row i of the logical [L, W]
    # grid holds the skew-shifted probability row i
    scr = [
        nc.dram_tensor(f"scr{b}", [L * W], FP16, kind="Internal") for b in range(B)
    ]

    with tile.TileContext(nc) as tc:
        with contextlib.ExitStack() as ctx:
            singles = ctx.enter_context(tc.tile_pool(name="singles", bufs=1))
            small = ctx.enter_context(tc.tile_pool(name="small", bufs=opts["small_bufs"]))
            ew_pool = ctx.enter_context(tc.tile_pool(name="ew", bufs=opts["ew_bufs"]))
            rt_pool = ctx.enter_context(tc.tile_pool(name="rt", bufs=opts["rt_bufs"]))
            ps_pool = ctx.enter_context(tc.tile_pool(name="ps", bufs=2, space="PSUM"))
            prep_pool = ctx.enter_context(tc.tile_pool(name="prep", bufs=2, space="PSUM"))
            s_pool = ctx.enter_context(tc.tile_pool(name="spsum", bufs=1, space="PSUM"))
            post_pool = ctx.enter_context(tc.tile_pool(name="post", bufs=2))

            # exp table warm-up: no data deps, so it schedules immediately and
            # the single act-table load lands before any real exp
            warm = singles.tile([1, 8], FP32)
            nc.vector.memset(warm, 0.0)
            warm2 = singles.tile([1, 8], FP32)
            nc.scalar.activation(out=warm2, in_=warm,
                                 func=mybir.ActivationFunctionType.Exp)
            # PE p-state warm-up: ~3us of dummy matmuls so the prep/score
            # matmuls on the first-exp critical path run at full clock
            wml = singles.tile([128, 1], FP16)
            nc.gpsimd.memset(wml, 0.0)
            wmr = singles.tile([128, 256], FP16)
            nc.gpsimd.memset(wmr, 0.0)
            wps = prep_pool.tile([128, 512], FP32, tag="prep", name="wps")
            for _ in range(12):
                nc.tensor.matmul(
                    out=wps[0:1, 0:256], lhsT=wml[:, 0:1], rhs=wmr[:, :],
                    start=True, stop=True,
                )

            prev_rd = {}
            # row 0 of the S accumulator finalizes column-span [a, b) after
            # block rr: block rr+1 only rewrites cols [1, rlen+1)
            ROW0_PIECES = {12: (385, 514), 13: (257, 385),
                           14: (129, 257), 15: (0, 129)}

            def emit_prep(b, weights):
                """QT/KT [32, L] fp16 and gate tile WB [128, L] fp16 for b.

                `slot` maps a global 512-col psum chunk index into the shared
                6-slot score region.  K is computed/copied first per chunk:
                the first score matmul needs both KT chunks of half 0 but
                only the first QT chunk, so K is the critical path.
                """
                bqk_sb, wqk_sb, xp_t = weights
                wq_sb = wqk_sb[:, 0:HD]
                wk_sb = wqk_sb[:, HD : 2 * HD]

                qt = singles.tile([HD, L], FP16, tag=f"QT{b}")
                kt = singles.tile([HD, L], FP16, tag=f"KT{b}")
                # b0's kt copies go on ACT (idle until the first exp); b1's
                # stay on DVE so they don't delay the early exp stream
                # packed prep psum: Q at partitions [0:32), K at [32:64)
                for ch in range(4):
                    kt_act = opts["kt_act"] or (
                        b == 0 and (opts["kt_act_b0"] or ch < 2)
                    )
                    cs = slice(ch * 512, (ch + 1) * 512)
                    pk = prep_pool.tile([128, 512], FP32, tag="prep")
                    # Q and K projections in one matmul: lhsT = [wq | wk]
                    # puts Q at partitions [0:32) and K at [32:64)
                    nc.tensor.matmul(
                        out=pk[0 : 2 * HD, :], lhsT=wqk_sb[:, :],
                        rhs=xp_t[:, cs], start=True, stop=True,
                    )
                    if opts["zero_bias"]:
                        if kt_act:
                            nc.scalar.copy(out=kt[:, cs], in_=pk[HD : 2 * HD, :])
                        else:
                            nc.vector.tensor_copy(
                                out=kt[:, cs], in_=pk[HD : 2 * HD, :]
                            )
                    else:
                        if kt_act:
                            nc.scalar.add(
                                out=kt[:, cs],
                                in_=pk[HD : 2 * HD, :],
                                add=bqk_sb[HD : 2 * HD],
                            )
                        else:
                            nc.vector.tensor_scalar_add(
                                out=kt[:, cs],
                                in0=pk[HD : 2 * HD, :],
                                scalar1=bqk_sb[HD : 2 * HD],
                            )
                    if opts["zero_bias"]:
                        nc.vector.tensor_copy(out=qt[:, cs], in_=pk[0:HD, :])
                    else:
                        nc.vector.tensor_scalar_add(
                            out=qt[:, cs],
                            in0=pk[0:HD, :],
                            scalar1=bqk_sb[0:HD],
                        )
                # gate tile arrives pre-broadcast from the host
                wb = singles.tile([128, L], FP16, tag=f"WB{b}")
                for half in range(2):
                    c0 = half * 1024
                    nc.sync.dma_start(
                        out=wb[:, c0 : c0 + 1024], in_=wbrev[b, :, c0 : c0 + 1024]
                    )
                return qt, kt, wb

            def emit_post_row(b, j, st, u, res):
                """Pool one 512-col segment (row at partition 32j).

                st[32j, t] = S'[512j + t - 1] (haloed; t in [0, 514)), so
                    u[t]   = st[t] + st[t+1]
                    res[c] = u[c] + st[c+2]  -> pooled[512j + c]
                All ops stay on partition 32j (engine ops need matching
                quadrant start partitions).
                """
                p = slice(32 * j, 32 * j + 1)
                nc.gpsimd.tensor_add(
                    out=u[p, 0:513], in0=st[p, 0:513], in1=st[p, 1:514]
                )
                nc.gpsimd.tensor_add(
                    out=res[p, 0:512], in0=u[p, 0:512], in1=st[p, 2:514]
                )

            def emit_instance():
                # b0's xp load is the critical path to the first exp: issue
                # it before the (tiny) weight loads so it leads the DMA queue
                xps = {}
                for b in range(B):
                    xps[b] = singles.tile([D, L], FP16, tag=f"xpT{b}", name="xp_t")
                nc.sync.dma_start(out=xps[0][:, 0:512], in_=xpT[0, :, 0:512])
                nc.sync.dma_start(out=xps[0][:, 512:1024], in_=xpT[0, :, 512:1024])
                wqk_sb = singles.tile([D, 2 * HD], FP16, tag="wqk_sb")
                nc.sync.dma_start(out=wqk_sb, in_=wqk[:, :])
                bqk_sb = singles.tile([2 * HD, 1], FP32, tag="bqk_sb")
                if not opts["zero_bias"]:
                    nc.sync.dma_start(out=bqk_sb, in_=bqk[:, :])
                nc.sync.dma_start(out=xps[0][:, 1024:L], in_=xpT[0, :, 1024:L])
                for c0, c1 in ((0, 1024), (1024, L)):
                    nc.sync.dma_start(out=xps[1][:, c0:c1], in_=xpT[1, :, c0:c1])

                QT, KT, WB = {}, {}, {}
                REDUCE_DELAY = opts["reduce_delay"]

                QT[0], KT[0], WB[0] = emit_prep(0, (bqk_sb, wqk_sb, xps[0]))
                QT[1], KT[1], WB[1] = emit_prep(1, (bqk_sb, wqk_sb, xps[1]))

                for b in range(B):
                    # haloed psum accumulator: row 32j col t holds
                    # S'[512j + t - 1] for t in [0, 514); the never-written
                    # edge cols (row0 t=0, row3 t=513) are zeroed up front
                    s4 = s_pool.tile([97, 516], FP32, tag="S4")
                    nc.vector.memset(s4[0:1, 0:1], 0.0)
                    nc.vector.memset(s4[96:97, 513:514], 0.0)
                    st = post_pool.tile([97, 514], FP32, tag="St")
                    u4 = post_pool.tile([97, 513], FP32, tag="u4")
                    res = post_pool.tile([97, 512], FP32, tag="res")
                    # row j's psum group closes one block after segment j's
                    # data cols (the 1-col left-halo tail), except row 0
                    rmax = [15, 12, 8, 4]
                    pending = []

                    def emit_reduce(item, s4=s4, st=st, u4=u4, res=res,
                                    rmax=rmax, b=b):
                        rr, rt_t, dr16, rlen_r = item
                        for j in range(4):
                            g0 = 512 * j - 1      # global col of local t=0
                            glo = max(0, g0)
                            ghi = min(rlen_r, 512 * j + 513)
                            if ghi <= glo:
                                continue
                            t0 = glo - g0
                            t1 = ghi - g0
                            # split at the psum bank boundary (col 512): each
                            # bank must hold exactly one accumulation group,
                            # stopping at the last block that writes it (this
                            # also keeps the moving free dim <= 512)
                            cuts = [t0]
                            if t0 < 512 < t1:
                                cuts.append(512)
                            cuts.append(t1)
                            for c0_, c1_ in zip(cuts[:-1], cuts[1:]):
                                nc.tensor.matmul(
                                    out=s4[32 * j : 32 * j + 1, c0_:c1_],
                                    lhsT=dr16[:, 0:1],
                                    rhs=rt_t[:, g0 + c0_ : g0 + c1_],
                                    start=(rr == 0),
                                    stop=(rr == (2047 - (g0 + c0_)) // 128),
                                    tile_position=(0, 32 * j),
                                )
                            if j == 0:
                                # row 0 finalizes col-span [a, bnd) after
                                # block rr (later blocks only rewrite lower
                                # cols), so pool it piecewise: the tail only
                                # keeps [0, 129)
                                pieces = (ROW0_PIECES if opts["row0_piece"] else {15: (0, 514)})
                                if rr in pieces:
                                    a, bnd = pieces[rr]
                                    eng = nc.vector if rr == 15 else nc.gpsimd
                                    p = slice(0, 1)
                                    nc.vector.tensor_copy(
                                        out=st[p, a:bnd], in_=s4[p, a:bnd]
                                    )
                                    bu = min(bnd, 513)
                                    eng.tensor_add(
                                        out=u4[p, a:bu],
                                        in0=st[p, a:bu],
                                        in1=st[p, a + 1 : bu + 1],
                                    )
                                    br = min(bnd, 512)
                                    eng.tensor_add(
                                        out=res[p, a:br],
                                        in0=u4[p, a:br],
                                        in1=st[p, a + 2 : br + 2],
                                    )
                            elif rr == rmax[j]:
                                # row j closed: copy to SBUF and pool (hidden
                                # off the tail for rows 1..3)
                                p = slice(32 * j, 32 * j + 1)
                                nc.vector.tensor_copy(
                                    out=st[p, :], in_=s4[p, 0:514]
                                )
                                emit_post_row(b, j, st, u4, res)

                    for r in range(NBLK):
                        i0 = r * 128
                        ew = ew_pool.tile([128, W], FP16, tag="ew")
                        if opts["memset_pool"]:
                            nc.gpsimd.memset(ew[:, L:W], 0.0)
                        else:
                            nc.vector.memset(ew[:, L:W], 0.0)
                        dcol = []
                        for half in range(2):
                            c0 = half * 1024
                            ps = ps_pool.tile([128, 1024], FP32, tag="ps")
                            for j in range(2):
                                nc.tensor.matmul(
                                    out=ps[:, j * 512 : (j + 1) * 512],
                                    lhsT=QT[b][:, i0 : i0 + 128],
                                    rhs=KT[b][:, c0 + j * 512 : c0 + (j + 1) * 512],
                                    start=True,
                                    stop=True,
                                )
                            dc = small.tile([128, 1], FP32, tag="dc")
                            nc.scalar.activation(
                                out=ew[:, c0 : c0 + 1024],
                                in_=ps[:, :],
                                func=mybir.ActivationFunctionType.Exp,
                                scale=SCALE,
                                accum_out=dc,
                            )
                            dcol.append(dc)
                        dsum = small.tile([128, 1], FP32, tag="ds")
                        nc.vector.tensor_add(out=dsum, in0=dcol[0], in1=dcol[1])
                        dr32 = small.tile([128, 1], FP32, tag="dr32")
                        nc.vector.reciprocal_approx_fast(out=dr32, in_=dsum)
                        drecip16 = small.tile([128, 1], FP16, tag="dr16")
                        with nc.allow_low_precision("1/(3d) in fp16; washes out"):
                            nc.vector.tensor_scalar_mul(
                                out=drecip16, in0=dr32, scalar1=1.0 / 3.0
                            )

                        # gate by w (columns i0..L of this block: upper triangle)
                        nc.vector.tensor_mul(
                            out=ew[:, i0:L],
                            in0=ew[:, i0:L],
                            in1=WB[b][:, i0:L],
                        )

                        # skewed write: scratch[i0+p, c - i0 - p] = ew[p, c]
                        # flat dst = i0*W + p*(W-1) + (c - i0)
                        wlen = (L + 127) - i0
                        dst = bass.AP(
                            tensor=scr[b],
                            offset=i0 * W,
                            ap=[[W - 1, 128], [1, wlen]],
                        )
                        weng = nc.gpsimd if opts["write_gpsimd"] else nc.sync
                        wr = weng.dma_start(out=dst, in_=ew[:, i0 : L + 127])
                        if (b, r) in prev_rd:
                            add_dep_helper(
                                wr.ins, prev_rd[(b, r)], True,
                                "scr WAR vs prev rep",
                            )

                        # plain read back: rows i0..i0+128, cols 0..L-i0
                        rlen = L - i0
                        src = bass.AP(
                            tensor=scr[b],
                            offset=i0 * W,
                            ap=[[W, 128], [1, rlen]],
                        )
                        rt = rt_pool.tile([128, L], FP16, tag="rt")
                        reng = nc.scalar if opts["read_act"] else nc.sync
                        rd = reng.dma_start(out=rt[:, 0:rlen], in_=src)
                        add_dep_helper(rd.ins, wr.ins, True, "scratch RAW")
                        prev_rd[(b, r)] = rd.ins

                        # S'[m] += sum_p (1/(3 d[i0+p])) * rt[p, m] -- emitted a
                        # few blocks late so PE isn't head-of-line blocked on
                        # the scratch round-trip
                        pending.append((r, rt, drecip16, rlen))
                        if len(pending) > REDUCE_DELAY:
                            emit_reduce(pending.pop(0))

                    while pending:
                        emit_reduce(pending.pop(0))

                    nc.sync.dma_start(out=out[b, :], in_=res[0:97:32, :])

            for _rep in range(repeat):
                emit_instance()

    nc.finalize()
    return nc


_RUNNERS = {}


def _get_runner(repeat=1, **opts_kw):
    key = (repeat, tuple(sorted(opts_kw.items())))
    if key in _RUNNERS:
        return _RUNNERS[key]
    import jax
    from jax.experimental.shard_map import shard_map
    from jax.sharding import Mesh, PartitionSpec

    from concourse import bass2jax

    nc = build_nc(repeat, **opts_kw)
    bass2jax.install_neuronx_cc_hook()

    partition_name = nc.partition_id_tensor.name if nc.partition_id_tensor else None
    in_names, out_names, out_avals = [], [], []
    for alloc in nc.m.functions[0].allocations:
        if not isinstance(alloc, mybir.MemoryLocationSet):
            continue
        name = alloc.memorylocations[0].name
        if alloc.kind == "ExternalInput":
            if name != partition_name:
                in_names.append(name)
        elif alloc.kind == "ExternalOutput":
            out_names.append(name)
            out_avals.append(
                jax.core.ShapedArray(
                    tuple(alloc.tensor_shape), mybir.dt.np(alloc.dtype)
                )
            )
    n_params = len(in_names)
    n_outs = len(out_names)
    all_in = list(in_names) + list(out_names)
    if partition_name is not None:
        all_in.append(partition_name)

    def _body(*args):
        operands = list(args)
        if partition_name is not None:
            operands.append(bass2jax.partition_id_tensor())
        outs = bass2jax._bass_exec_p.bind(
            *operands,
            out_avals=tuple(out_avals),
            in_names=tuple(all_in),
            out_names=tuple(out_names),
            lowering_input_output_aliases=(),
            sim_require_finite=True,
            sim_require_nnan=True,
            nc=nc,
        )
        return tuple(outs)

    devices = jax.devices()[:H]
    mesh = Mesh(np.asarray(devices), ("core",))
    sharded = jax.jit(
        shard_map(
            _body,
            mesh=mesh,
            in_specs=(PartitionSpec("core"),) * (n_params + n_outs),
            out_specs=(PartitionSpec("core"),) * n_outs,
            check_rep=False,
        ),
        donate_argnums=tuple(range(n_params, n_params + n_outs)),
        keep_unused=True,
    )
    runner = (sharded, in_names, out_names, out_avals)
    _RUNNERS[key] = runner
    return runner


def _prep_in_maps(x, pe, Wq, bq, Wk, bk, Wv):
    x = np.asarray(x, np.float32)
    pe = np.asarray(pe, np.float32)
    Wq = np.asarray(Wq, np.float32)
    bq = np.asarray(bq, np.float32)
    Wk = np.asarray(Wk, np.float32)
    bk = np.asarray(bk, np.float32)
    Wv = np.asarray(Wv, np.float32)

    xp = x + pe[None, :, :]
    xpT = np.ascontiguousarray(xp.transpose(0, 2, 1)).astype(np.float16)
    # gate w[k] = sigmoid(x @ Wv)[L-1-k, h], broadcast to 128 partitions
    vlog = np.einsum("bld,dh->blh", x.astype(np.float64), Wv.astype(np.float64))
    vsig = 1.0 / (1.0 + np.exp(-vlog))          # (B, L, H)
    vrev = vsig[:, ::-1, :]                      # (B, L, H) reversed over l

    in_maps = []
    for h in range(H):
        hs = slice(h * HD, (h + 1) * HD)
        bqk = np.concatenate([bq[hs], bk[hs]]).reshape(2 * HD, 1)
        wb = np.ascontiguousarray(
            np.broadcast_to(vrev[:, None, :, h], (B, 128, L))
        ).astype(np.float16)
        wqk = np.concatenate([Wq[:, hs], Wk[:, hs]], axis=1)
        in_maps.append(
            {
                "xpT": xpT,
                "wbrev": wb,
                "wqk": np.ascontiguousarray(wqk).astype(np.float16),
                "bqk": np.ascontiguousarray(bqk).astype(np.float32),
            }
        )
    return in_maps


def run(in_maps, repeat=1, **opts_kw):
    sharded, in_names, out_names, out_avals = _get_runner(repeat, **opts_kw)
    concat_in = [
        np.concatenate([np.asarray(in_maps[c][n]) for c in range(H)], axis=0)
        for n in in_names
    ]
    concat_zeros = [
        np.zeros((H * a.shape[0], *a.shape[1:]), a.dtype) for a in out_avals
    ]
    out_arrs = sharded(*concat_in, *concat_zeros)
    return [
        {
            n: np.asarray(out_arrs[i]).reshape(H, *out_avals[i].shape)[c]
            for i, n in enumerate(out_names)
        }
        for c in range(H)
    ]


def kernel(x, pe, Wq, bq, Wk, bk, Wv):
    in_maps = _prep_in_maps(x, pe, Wq, bq, Wk, bk, Wv)
    zb = not (np.any(np.asarray(bq)) or np.any(np.asarray(bk)))
    results = run(in_maps, repeat=1, zero_bias=bool(zb))
    res = np.stack([results[h]["out"] for h in range(H)], axis=2)
    # window-3 SAME avg-pool divisor: ends divide by 2, not 3 (the kernel
    # pre-folds 1/3 into the reduce weights, so the two edge columns need
    # a 3/2 correction)
    res[:, 0, :] *= 1.5
    res[:, L - 1, :] *= 1.5
    return res


# revision 54
# speedup vs baseline: 1.2560x; 1.0007x over previous
"""Trainium2 Bass kernel for nn_MultiHeadDistanceLayer (sparse_attention).

Math: for each (head h, batch b) the reference collapses to
    S[m] = sum_k attn[k-m, k] * w[k],   w[k] = sigmoid(x @ Wv)[L-1-k, h]
(weighted superdiagonal sums of the attention matrix), followed by a
window-3 same-padded average pool over m (padding excluded from the
divisor):  out[b, m, h] = (S[m-1] + S[m] + S[m+1]) / cnt[m].

Sharding: 8 heads -> 8 NeuronCores; each core computes its head for both
batches.  Per (h, b) the kernel runs flash-style over 16 row blocks of 128
queries: scores via PE (K=HD=32), exp on ACT (with free row-sum accum ->
softmax denominators d), gating by w on DVE, then a *skewed* DMA write of
the probability block to a DRAM scratch so that superdiagonal m lands at
column m of every row.  A plain strided read back + a [1/(3d)]-weighted
ones-matmul on PE reduces partitions, PSUM-accumulating S/3 into a single
PSUM bank laid out as 4 rows at partitions {0,32,64,96} (512 cols each).
The window-3 pool then runs on DVE as 4-partition-parallel strided ops
(the /3 divisor is pre-folded; the two boundary elements get *1.5).
The sigmoid gate w is precomputed on the host (O(L*D) prep, like x+pe)
and arrives pre-broadcast as a [128, L] tile per batch, which removes the
sigmoid/exp ACT-table swap and the reversed-x input entirely.
"""

import contextlib

import numpy as np

import concourse.bacc as bacc
import concourse.bass as bass
import concourse.tile as tile
from concourse import mybir
from concourse.tile import add_dep_helper

B, L, D, H, HD, WIN = 2, 2048, 128, 8, 32, 3
NBLK = L // 128           # 16 row blocks per (h, b)
W = L + 128               # scratch row width (elements)
SCALE = float(HD) ** -0.5

FP16 = mybir.dt.float16
FP32 = mybir.dt.float32

DEFAULT_OPTS = dict(
    ew_bufs=10,
    rt_bufs=10,
    zero_bias=False,     # biases known to be zero -> plain copy instead of add
    kt_act=False,        # prep K-copies on DVE (ACT stays exp-only)
    reduce_delay=7,
    memset_pool=False,   # ew tail memset on DVE
    small_bufs=8,
    read_act=False,      # scratch read-back DMAs issued from ACT queue
    kt_act_b0=False,     # b0 prep K-copies on ACT
    row0_piece=True,     # piecewise row-0 pool
    write_gpsimd=False,  # scratch writes via SWDGE (Pool) instead of HWDGE
)


def build_nc(repeat=1, **opts_kw):
    opts = dict(DEFAULT_OPTS, **opts_kw)
    nc = bacc.Bacc("TRN2", target_bir_lowering=False, debug=False)

    xpT = nc.dram_tensor("xpT", [B, D, L], FP16, kind="ExternalInput")
    wbrev = nc.dram_tensor("wbrev", [B, 128, L], FP16, kind="ExternalInput")
    wqk = nc.dram_tensor("wqk", [D, 2 * HD], FP16, kind="ExternalInput")
    bqk = nc.dram_tensor("bqk", [2 * HD, 1], FP32, kind="ExternalInput")
    out = nc.dram_tensor("out", [B, L], FP32, kind="ExternalOutput")
    # one flat scratch region per (h, b) pair; row i of the logical [L, W]
    # grid holds the skew-shifted probability row i
    scr = [
        nc.dram_tensor(f"scr{b}", [L * W], FP16, kind="Internal") for b in range(B)
    ]

    with tile.TileContext(nc) as tc:
        with contextlib.ExitStack() as ctx:
            singles = ctx.enter_context(tc.tile_pool(name="singles", bufs=1))
            small = ctx.enter_context(tc.tile_pool(name="small", bufs=opts["small_bufs"]))
            ew_pool = ctx.enter_context(tc.tile_pool(name="ew", bufs=opts["ew_bufs"]))
            rt_pool = ctx.enter_context(tc.tile_pool(name="rt", bufs=opts["rt_bufs"]))
            ps_pool = ctx.enter_context(tc.tile_pool(name="ps", bufs=2, space="PSUM"))
            prep_pool = ctx.enter_context(tc.tile_pool(name="prep", bufs=2, space="PSUM"))
            s_pool = ctx.enter_context(tc.tile_pool(name="spsum", bufs=1, space="PSUM"))
            post_pool = ctx.enter_context(tc.tile_pool(name="post", bufs=2))

            # exp table warm-up: no data deps, so it schedules immediately and
            # the single act-table load lands before any real exp
            warm = singles.tile([1, 8], FP32)
            nc.vector.memset(warm, 0.0)
            warm2 = singles.tile([1, 8], FP32)
            nc.scalar.activation(out=warm2, in_=warm,
                                 func=mybir.ActivationFunctionType.Exp)
            # PE p-state warm-up: ~3us of dummy matmuls so the prep/score
            # matmuls on the first-exp critical path run at full clock
            wml = singles.tile([128, 1], FP16)
            nc.gpsimd.memset(wml, 0.0)
            wmr = singles.tile([128, 256], FP16)
            nc.gpsimd.memset(wmr, 0.0)
            wps = prep_pool.tile([128, 512], FP32, tag="prep", name="wps")
            for _ in range(12):
                nc.tensor.matmul(
                    out=wps[0:1, 0:256], lhsT=wml[:, 0:1], rhs=wmr[:, :],
                    start=True, stop=True,
                )

            prev_rd = {}
            # row 0 of the S accumulator finalizes column-span [a, b) after
            # block rr: block rr+1 only rewrites cols [1, rlen+1)
            ROW0_PIECES = {12: (385, 514), 13: (257, 385),
                           14: (129, 257), 15: (0, 129)}

            def emit_prep(b, weights):
                """QT/KT [32, L] fp16 and gate tile WB [128, L] fp16 for b.

                `slot` maps a global 512-col psum chunk index into the shared
                6-slot score region.  K is computed/copied first per chunk:
                the first score matmul needs both KT chunks of half 0 but
                only the first QT chunk, so K is the critical path.
                """
                bqk_sb, wqk_sb, xp_t = weights
                wq_sb = wqk_sb[:, 0:HD]
                wk_sb = wqk_sb[:, HD : 2 * HD]

                qt = singles.tile([HD, L], FP16, tag=f"QT{b}")
                kt = singles.tile([HD, L], FP16, tag=f"KT{b}")
                # b0's kt copies go on ACT (idle until the first exp); b1's
                # stay on DVE so they don't delay the early exp stream
                # packed prep psum: Q at partitions [0:32), K at [32:64)
                for ch in range(4):
                    kt_act = opts["kt_act"] or (
                        b == 0 and (opts["kt_act_b0"] or ch < 2)
                    )
                    cs = slice(ch * 512, (ch + 1) * 512)
                    pk = prep_pool.tile([128, 512], FP32, tag="prep")
                    # Q and K projections in one matmul: lhsT = [wq | wk]
                    # puts Q at partitions [0:32) and K at [32:64)
                    nc.tensor.matmul(
                        out=pk[0 : 2 * HD, :], lhsT=wqk_sb[:, :],
                        rhs=xp_t[:, cs], start=True, stop=True,
                    )
                    if opts["zero_bias"]:
                        if kt_act:
                            nc.scalar.copy(out=kt[:, cs], in_=pk[HD : 2 * HD, :])
                        else:
                            nc.vector.tensor_copy(
                                out=kt[:, cs], in_=pk[HD : 2 * HD, :]
                            )
                    else:
                        if kt_act:
                            nc.scalar.add(
                                out=kt[:, cs],
                                in_=pk[HD : 2 * HD, :],
                                add=bqk_sb[HD : 2 * HD],
                            )
                        else:
                            nc.vector.tensor_scalar_add(
                                out=kt[:, cs],
                                in0=pk[HD : 2 * HD, :],
                                scalar1=bqk_sb[HD : 2 * HD],
                            )
                    if opts["zero_bias"]:
                        nc.vector.tensor_copy(out=qt[:, cs], in_=pk[0:HD, :])
                    else:
                        nc.vector.tensor_scalar_add(
                            out=qt[:, cs],
                            in0=pk[0:HD, :],
                            scalar1=bqk_sb[0:HD],
                        )
                # gate tile arrives pre-broadcast from the host
                wb = singles.tile([128, L], FP16, tag=f"WB{b}")
                for half in range(2):
                    c0 = half * 1024
                    nc.sync.dma_start(
                        out=wb[:, c0 : c0 + 1024], in_=wbrev[b, :, c0 : c0 + 1024]
                    )
                return qt, kt, wb

            def emit_post_row(b, j, st, u, res):
                """Pool one 512-col segment (row at partition 32j).

                st[32j, t] = S'[512j + t - 1] (haloed; t in [0, 514)), so
                    u[t]   = st[t] + st[t+1]
                    res[c] = u[c] + st[c+2]  -> pooled[512j + c]
                All ops stay on partition 32j (engine ops need matching
                quadrant start partitions).
                """
                p = slice(32 * j, 32 * j + 1)
                nc.gpsimd.tensor_add(
                    out=u[p, 0:513], in0=st[p, 0:513], in1=st[p, 1:514]
                )
                nc.gpsimd.tensor_add(
                    out=res[p, 0:512], in0=u[p, 0:512], in1=st[p, 2:514]
                )

            def emit_instance():
                # b0's xp load is the critical path to the first exp: issue
                # it before the (tiny) weight loads so it leads the DMA queue
                xps = {}
                for b in range(B):
                    xps[b] = singles.tile([D, L], FP16, tag=f"xpT{b}", name="xp_t")
                nc.sync.dma_start(out=xps[0][:, 0:512], in_=xpT[0, :, 0:512])
                nc.sync.dma_start(out=xps[0][:, 512:1024], in_=xpT[0, :, 512:1024])
                wqk_sb = singles.tile([D, 2 * HD], FP16, tag="wqk_sb")
                nc.sync.dma_start(out=wqk_sb, in_=wqk[:, :])
                bqk_sb = singles.tile([2 * HD, 1], FP32, tag="bqk_sb")
                if not opts["zero_bias"]:
                    nc.sync.dma_start(out=bqk_sb, in_=bqk[:, :])
                nc.sync.dma_start(out=xps[0][:, 1024:L], in_=xpT[0, :, 1024:L])
                for c0, c1 in ((0, 1024), (1024, L)):
                    nc.sync.dma_start(out=xps[1][:, c0:c1], in_=xpT[1, :, c0:c1])

                QT, KT, WB = {}, {}, {}
                REDUCE_DELAY = opts["reduce_delay"]

                QT[0], KT[0], WB[0] = emit_prep(0, (bqk_sb, wqk_sb, xps[0]))
                QT[1], KT[1], WB[1] = emit_prep(1, (bqk_sb, wqk_sb, xps[1]))

                for b in range(B):
                    # haloed psum accumulator: row 32j col t holds
                    # S'[512j + t - 1] for t in [0, 514); the never-written
                    # edge cols (row0 t=0, row3 t=513) are zeroed up front
                    s4 = s_pool.tile([97, 516], FP32, tag="S4")
                    nc.vector.memset(s4[0:1, 0:1], 0.0)
                    nc.vector.memset(s4[96:97, 513:514], 0.0)
                    st = post_pool.tile([97, 514], FP32, tag="St")
                    u4 = post_pool.tile([97, 513], FP32, tag="u4")
                    res = post_pool.tile([97, 512], FP32, tag="res")
                    # row j's psum group closes one block after segment j's
                    # data cols (the 1-col left-halo tail), except row 0
                    rmax = [15, 12, 8, 4]
                    pending = []

                    def emit_reduce(item, s4=s4, st=st, u4=u4, res=res,
                                    rmax=rmax, b=b):
                        rr, rt_t, dr16, rlen_r = item
                        for j in range(4):
                            g0 = 512 * j - 1      # global col of local t=0
                            glo = max(0, g0)
                            ghi = min(rlen_r, 512 * j + 513)
                            if ghi <= glo:
                                continue
                            t0 = glo - g0
                            t1 = ghi - g0
                            # split at the psum bank boundary (col 512): each
                            # bank must hold exactly one accumulation group,
                            # stopping at the last block that writes it (this
                            # also keeps the moving free dim <= 512)
                            cuts = [t0]
                            if t0 < 512 < t1:
                                cuts.append(512)
                            cuts.append(t1)
                            for c0_, c1_ in zip(cuts[:-1], cuts[1:]):
                                nc.tensor.matmul(
                                    out=s4[32 * j : 32 * j + 1, c0_:c1_],
                                    lhsT=dr16[:, 0:1],
                                    rhs=rt_t[:, g0 + c0_ : g0 + c1_],
                                    start=(rr == 0),
                                    stop=(rr == (2047 - (g0 + c0_)) // 128),
                                    tile_position=(0, 32 * j),
                                )
                            if j == 0:
                                # row 0 finalizes col-span [a, bnd) after
                                # block rr (later blocks only rewrite lower
                                # cols), so pool it piecewise: the tail only
                                # keeps [0, 129)
                                pieces = (ROW0_PIECES if opts["row0_piece"] else {15: (0, 514)})
                                if rr in pieces:
                                    a, bnd = pieces[rr]
                                    eng = nc.vector if rr == 15 else nc.gpsimd
                                    p = slice(0, 1)
                                    nc.vector.tensor_copy(
                                        out=st[p, a:bnd], in_=s4[p, a:bnd]
                                    )
                                    bu = min(bnd, 513)
                                    eng.tensor_add(
                                        out=u4[p, a:bu],
                                        in0=st[p, a:bu],
                                        in1=st[p, a + 1 : bu + 1],
                                    )
                                    br = min(bnd, 512)
                                    eng.tensor_add(
                                        out=res[p, a:br],
                                        in0=u4[p, a:br],
                                        in1=st[p, a + 2 : br + 2],
                                    )
                            elif rr == rmax[j]:
                                # row j closed: copy to SBUF and pool (hidden
                                # off the tail for rows 1..3)
                                p = slice(32 * j, 32 * j + 1)
                                nc.vector.tensor_copy(
                                    out=st[p, :], in_=s4[p, 0:514]
                                )
                                emit_post_row(b, j, st, u4, res)

                    for r in range(NBLK):
                        i0 = r * 128
                        ew = ew_pool.tile([128, W], FP16, tag="ew")
                        if opts["memset_pool"]:
                            nc.gpsimd.memset(ew[:, L:W], 0.0)
                        else:
                            nc.vector.memset(ew[:, L:W], 0.0)
                        dcol = []
                        for half in range(2):
                            c0 = half * 1024
                            ps = ps_pool.tile([128, 1024], FP32, tag="ps")
                            for j in range(2):
                                nc.tensor.matmul(
                                    out=ps[:, j * 512 : (j + 1) * 512],
                                    lhsT=QT[b][:, i0 : i0 + 128],
                                    rhs=KT[b][:, c0 + j * 512 : c0 + (j + 1) * 512],
                                    start=True,
                                    stop=True,
                                )
                            dc = small.tile([128, 1], FP32, tag="dc")
                            nc.scalar.activation(
                                out=ew[:, c0 : c0 + 1024],
                                in_=ps[:, :],
                                func=mybir.ActivationFunctionType.Exp,
                                scale=SCALE,
                                accum_out=dc,
                            )
                            dcol.append(dc)
                        dsum = small.tile([128, 1], FP32, tag="ds")
                        nc.vector.tensor_add(out=dsum, in0=dcol[0], in1=dcol[1])
                        dr32 = small.tile([128, 1], FP32, tag="dr32")
                        nc.vector.reciprocal_approx_fast(out=dr32, in_=dsum)
                        drecip16 = small.tile([128, 1], FP16, tag="dr16")
                        with nc.allow_low_precision("1/(3d) in fp16; washes out"):
                            nc.vector.tensor_scalar_mul(
                                out=drecip16, in0=dr32, scalar1=1.0 / 3.0
                            )

                        # gate by w (columns i0..L of this block: upper triangle)
                        nc.vector.tensor_mul(
                            out=ew[:, i0:L],
                            in0=ew[:, i0:L],
                            in1=WB[b][:, i0:L],
                        )

                        # skewed write: scratch[i0+p, c - i0 - p] = ew[p, c]
                        # flat dst = i0*W + p*(W-1) + (c - i0)
                        wlen = (L + 127) - i0
                        dst = bass.AP(
                            tensor=scr[b],
                            offset=i0 * W,
                            ap=[[W - 1, 128], [1, wlen]],
                        )
                        weng = nc.gpsimd if opts["write_gpsimd"] else nc.sync
                        wr = weng.dma_start(out=dst, in_=ew[:, i0 : L + 127])
                        if (b, r) in prev_rd:
                            add_dep_helper(
                                wr.ins, prev_rd[(b, r)], True,
                                "scr WAR vs prev rep",
                            )

                        # plain read back: rows i0..i0+128, cols 0..L-i0
                        rlen = L - i0
                        src = bass.AP(
                            tensor=scr[b],
                            offset=i0 * W,
                            ap=[[W, 128], [1, rlen]],
                        )
                        rt = rt_pool.tile([128, L], FP16, tag="rt")
                        reng = nc.scalar if opts["read_act"] else nc.sync
                        rd = reng.dma_start(out=rt[:, 0:rlen], in_=src)
                        add_dep_helper(rd.ins, wr.ins, True, "scratch RAW")
                        prev_rd[(b, r)] = rd.ins

                        # S'[m] += sum_p (1/(3 d[i0+p])) * rt[p, m] -- emitted a
                        # few blocks late so PE isn't head-of-line blocked on
                        # the scratch round-trip
                        pending.append((r, rt, drecip16, rlen))
                        if len(pending) > REDUCE_DELAY:
                            emit_reduce(pending.pop(0))

                    while pending:
                        emit_reduce(pending.pop(0))

                    nc.sync.dma_start(out=out[b, :], in_=res[0:97:32, :])

            for _rep in range(repeat):
                emit_instance()

    nc.finalize()
    return nc


_RUNNERS = {}


def _get_runner(repeat=1, **opts_kw):
    key = (repeat, tuple(sorted(opts_kw.items())))
    if key in _RUNNERS:
        return _RUNNERS[key]
    import jax
    from jax.experimental.shard_map import shard_map
    from jax.sharding import Mesh, PartitionSpec

    from concourse import bass2jax

    nc = build_nc(repeat, **opts_kw)
    bass2jax.install_neuronx_cc_hook()

    partition_name = nc.partition_id_tensor.name if nc.partition_id_tensor else None
    in_names, out_names, out_avals = [], [], []
    for alloc in nc.m.functions[0].allocations:
        if not isinstance(alloc, mybir.MemoryLocationSet):
            continue
        name = alloc.memorylocations[0].name
        if alloc.kind == "ExternalInput":
            if name != partition_name:
                in_names.append(name)
        elif alloc.kind == "ExternalOutput":
            out_names.append(name)
            out_avals.append(
                jax.core.ShapedArray(
                    tuple(alloc.tensor_shape), mybir.dt.np(alloc.dtype)
                )
            )
    n_params = len(in_names)
    n_outs = len(out_names)
    all_in = list(in_names) + list(out_names)
    if partition_name is not None:
        all_in.append(partition_name)

    def _body(*args):
        operands = list(args)
        if partition_name is not None:
            operands.append(bass2jax.partition_id_tensor())
        outs = bass2jax._bass_exec_p.bind(
            *operands,
            out_avals=tuple(out_avals),
            in_names=tuple(all_in),
            out_names=tuple(out_names),
            lowering_input_output_aliases=(),
            sim_require_finite=True,
            sim_require_nnan=True,
            nc=nc,
        )
        return tuple(outs)

    devices = jax.devices()[:H]
    mesh = Mesh(np.asarray(devices), ("core",))
    sharded = jax.jit(
        shard_map(
            _body,
            mesh=mesh,
            in_specs=(PartitionSpec("core"),) * (n_params + n_outs),
            out_specs=(PartitionSpec("core"),) * n_outs,
            check_rep=False,
        ),
        donate_argnums=tuple(range(n_params, n_params + n_outs)),
        keep_unused=True,
    )
    runner = (sharded, in_names, out_names, out_avals)
    _RUNNERS[key] = runner
    return runner


def _prep_in_maps(x, pe, Wq, bq, Wk, bk, Wv):
    x = np.asarray(x, np.float32)
    pe = np.asarray(pe, np.float32)
    Wq = np.asarray(Wq, np.float32)
    bq = np.asarray(bq, np.float32)
    Wk = np.asarray(Wk, np.float32)
    bk = np.asarray(bk, np.float32)
    Wv = np.asarray(Wv, np.float32)

    xp = x + pe[None, :, :]
    xpT = np.ascontiguousarray(xp.transpose(0, 2, 1)).astype(np.float16)
    # gate w[k] = sigmoid(x @ Wv)[L-1-k, h], broadcast to 128 partitions
    vlog = np.einsum("bld,dh->blh", x.astype(np.float64), Wv.astype(np.float64))
    vsig = 1.0 / (1.0 + np.exp(-vlog))          # (B, L, H)
    vrev = vsig[:, ::-1, :]                      # (B, L, H) reversed over l

    in_maps = []
    for h in range(H):
        hs = slice(h * HD, (h + 1) * HD)
        bqk = np.concatenate([bq[hs], bk[hs]]).reshape(2 * HD, 1)
        wb = np.ascontiguousarray(
            np.broadcast_to(vrev[:, None, :, h], (B, 128, L))
        ).astype(np.float16)
        wqk = np.concatenate([Wq[:, hs], Wk[:, hs]], axis=1)
        in_maps.append(
            {
                "xpT": xpT,
                "wbrev": wb,
                "wqk": np.ascontiguousarray(wqk).astype(np.float16),
                "bqk": np.ascontiguousarray(bqk).astype(np.float32),
            }
        )
    return in_maps


def run(in_maps, repeat=1, **opts_kw):
    sharded, in_names, out_names, out_avals = _get_runner(repeat, **opts_kw)
    concat_in = [
        np.concatenate([np.asarray(in_maps[c][n]) for c in range(H)], axis=0)
        for n in in_names
    ]
    concat_zeros = [
        np.zeros((H * a.shape[0], *a.shape[1:]), a.dtype) for a in out_avals
    ]
    out_arrs = sharded(*concat_in, *concat_zeros)
    return [
        {
            n: np.asarray(out_arrs[i]).reshape(H, *out_avals[i].shape)[c]
            for i, n in enumerate(out_names)
        }
        for c in range(H)
    ]


def kernel(x, pe, Wq, bq, Wk, bk, Wv):
    in_maps = _prep_in_maps(x, pe, Wq, bq, Wk, bk, Wv)
    zb = not (np.any(np.asarray(bq)) or np.any(np.asarray(bk)))
    results = run(in_maps, repeat=1, zero_bias=bool(zb))
    res = np.stack([results[h]["out"] for h in range(H)], axis=2)
    # window-3 SAME avg-pool divisor: ends divide by 2, not 3 (the kernel
    # pre-folds 1/3 into the reduce weights, so the two edge columns need
    # a 3/2 correction)
    res[:, 0, :] *= 1.5
    res[:, L - 1, :] *= 1.5
    return res
